# revision 12
# baseline (speedup 1.0000x reference)
"""Trainium2 Bass kernel for CompositionalTwoArmedAgent (DND-LSTM A2C step).

Strategy (8 NeuronCores, SPMD + AllReduce):
  - DND keys/vals tables sharded row-wise: 12544 rows/core (core 7 padded).
  - Cosine similarities are bounded in [-1, 1], so the softmax needs no
    max pass: each core computes e_i = exp(cos_i), a partial sum S_k and a
    partial weighted value sum p_k = e @ vals_k (TensorE, fp32r full rate).
  - The LSTM i2h/h2h GEMM is sharded over its contraction dim (128 h-dims
    per core; the x_t @ W_i2h.T part is zero-fed on cores 1..7).
  - Two AllReduces: [preact(5120) | S(1)] early (hidden under the vals
    stream, so the gate math is precomputed), p(1024) late (4 KB).
  - Every core then computes the identical tiny LSTM/A2C tail; host reads
    core 0's output, applies the 2-class softmax / fixed-key categorical
    sample, and packs the reference's output layout.
"""

import ml_dtypes
import numpy as np

import concourse.bacc as bacc
import concourse.bass as bass
import concourse.mybir as mybir
import concourse.tile as tile
from concourse.bass_utils import run_bass_kernel_spmd

N_CORES = 8
D, RD, H, IN_DIM, A = 100000, 10, 1024, 14, 2
PER = 12544            # padded rows per core = G * 128
G = 98                 # 128-row chunks per core
BLOCKS = [7] * 13 + [4, 2, 1]   # chunks per vals DMA block (descending tail)
F32 = mybir.dt.float32
F32R = mybir.dt.float32r
BF16 = mybir.dt.bfloat16

# jax.random.gumbel(jax.random.key(1), (2,), float32) — fixed constants of the
# reference's categorical sample (verified against jax.random.categorical).
GUMBEL = np.array([0.5325072, -0.01641824], np.float32)

_CACHE = {}


def _input_specs():
    return [
        ("vals_s", [128, G * H], F32R),    # row-chunk-tiled vals shard
        ("keys_t", [128, G * RD], F32),
        ("q_rep", [128, G * RD], F32),
        ("mask", [128, G], F32),
        ("wht", [128, 5 * H], F32R),
        ("wxt", [IN_DIM, 5 * H], F32R),
        ("x_col", [IN_DIM, 1], F32R),
        ("h_col", [128, 1], F32R),
        ("c2t", [128, 8], F32),
        ("b5t", [128, 40], F32),
        ("biht", [128, 8], F32),
        ("wiht", [128, 8 * H], BF16),
        ("wact", [128, 24], F32),
        ("bac", [1, 3], F32),
    ]


def _build():
    nc = bacc.Bacc("TRN2", target_bir_lowering=False, debug=False,
                   num_devices=N_CORES)
    d = {name: nc.dram_tensor(name, shp, dt, kind="ExternalInput")
         for name, shp, dt in _input_specs()}
    out_hc = nc.dram_tensor("out_hc", [128, 16], F32, kind="ExternalOutput")
    out_av = nc.dram_tensor("out_av", [1, 3], F32, kind="ExternalOutput")

    AF = mybir.ActivationFunctionType
    OP = mybir.AluOpType

    with tile.TileContext(nc) as tc:
        with (
            tc.tile_pool(name="const", bufs=1) as cp,
            tc.tile_pool(name="vals", bufs=3) as vp,
            tc.tile_pool(name="ps", bufs=1, space="PSUM") as pp,
            tc.tile_pool(name="dram", bufs=1, space="DRAM") as dp,
        ):
            # ---- persistent loads -------------------------------------
            keys_sb = cp.tile([128, G * RD], F32)
            q_sb = cp.tile([128, G * RD], F32)
            mask_sb = cp.tile([128, G], F32)
            wht_sb = cp.tile([128, 5 * H], F32R)
            wxt_sb = cp.tile([IN_DIM, 5 * H], F32R)
            x_col_sb = cp.tile([IN_DIM, 1], F32R)
            h_col_sb = cp.tile([128, 1], F32R)
            c2t_sb = cp.tile([128, 8], F32)
            b5t_sb = cp.tile([128, 40], F32)
            biht_sb = cp.tile([128, 8], F32)
            wiht_sb = cp.tile([128, 8, H], BF16)
            wact_sb = cp.tile([128, 24], F32)
            bac_sb = cp.tile([1, 3], F32)
            for name, t in [("keys_t", keys_sb), ("q_rep", q_sb),
                            ("mask", mask_sb), ("wht", wht_sb),
                            ("wxt", wxt_sb), ("x_col", x_col_sb),
                            ("h_col", h_col_sb), ("c2t", c2t_sb),
                            ("b5t", b5t_sb), ("biht", biht_sb),
                            ("wact", wact_sb), ("bac", bac_sb)]:
                nc.scalar.dma_start(t[:], d[name][:])
            nc.scalar.dma_start(
                wiht_sb[:], d["wiht"][:].rearrange("p (c j) -> p c j", j=H))

            ones_sb = cp.tile([128, 128], F32)
            nc.vector.memset(ones_sb[:], 1.0)

            # ---- ||q||^2 broadcast to all partitions ------------------
            sq_q = cp.tile([1, RD], F32)
            nc.scalar.activation(sq_q[:], q_sb[0:1, 0:RD], AF.Square)
            qnsq = cp.tile([1, 1], F32)
            nc.vector.reduce_sum(qnsq[:], sq_q[:], axis=mybir.AxisListType.X)
            psum_qn = pp.tile([128, 1], F32, tag="ps_small")
            nc.tensor.matmul(psum_qn[:], ones_sb[0:1, :], qnsq[:])
            qn2b = cp.tile([128, 1], F32)
            nc.vector.tensor_copy(qn2b[:], psum_qn[:])

            # ---- cosine sims -> masked exp weights --------------------
            prod = cp.tile([128, G * RD], F32)
            nc.vector.tensor_mul(prod[:], keys_sb[:], q_sb[:])
            dots = cp.tile([128, G], F32)
            nc.vector.tensor_reduce(
                dots[:], prod[:].rearrange("p (g r) -> p g r", r=RD),
                axis=mybir.AxisListType.X, op=OP.add)
            sqk = cp.tile([128, G * RD], F32)
            nc.scalar.activation(sqk[:], keys_sb[:], AF.Square)
            nsq = cp.tile([128, G], F32)
            nc.vector.tensor_reduce(
                nsq[:], sqk[:].rearrange("p (g r) -> p g r", r=RD),
                axis=mybir.AxisListType.X, op=OP.add)
            d2 = cp.tile([128, G], F32)
            nc.vector.tensor_scalar(d2[:], nsq[:], qn2b[:, 0:1], None, OP.mult)
            den = cp.tile([128, G], F32)
            nc.scalar.activation(den[:], d2[:], AF.Sqrt)
            denc = cp.tile([128, G], F32)
            nc.vector.tensor_scalar_max(denc[:], den[:], 1e-8)
            rec = cp.tile([128, G], F32)
            nc.vector.reciprocal(rec[:], denc[:])
            s_sb = cp.tile([128, G], F32)
            nc.vector.tensor_mul(s_sb[:], dots[:], rec[:])
            eraw = cp.tile([128, G], F32)
            nc.scalar.activation(eraw[:], s_sb[:], AF.Exp)
            e_sb = cp.tile([128, G], F32)
            rowsum = cp.tile([128, 1], F32)
            nc.vector.scalar_tensor_tensor(
                e_sb[:], eraw[:], 1.0, mask_sb[:], OP.mult, OP.mult,
                accum_out=rowsum[:])
            e_r = cp.tile([128, G], F32R)
            nc.vector.tensor_copy(e_r[:], e_sb[:])

            # ---- preact partial: [x;h_chunk] @ [WxT;WhT] --------------
            # moving-operand fp32r form: 20 N=512 matmuls into [1,512] rows,
            # then 40 PE transposes back to the compact [128, 40] col layout.
            psum_pre = pp.tile([128, 40], F32)
            for n in range(10):
                pre_ps = pp.tile([1, 512], F32, tag=f"pre{n % 2}")
                nc.tensor.matmul(pre_ps[:], h_col_sb[:],
                                 wht_sb[:, n * 512:(n + 1) * 512],
                                 start=True, stop=False)
                nc.tensor.matmul(pre_ps[:], x_col_sb[:],
                                 wxt_sb[:, n * 512:(n + 1) * 512],
                                 start=False, stop=True)
                row_scr = cp.tile([1, 512], F32, tag="rowscr", bufs=2)
                nc.vector.tensor_copy(row_scr[:], pre_ps[:])
                for t in range(4):
                    nc.tensor.transpose(psum_pre[:, 4 * n + t:4 * n + t + 1],
                                        row_scr[0:1, t * 128:(t + 1) * 128],
                                        ones_sb[0:1, 0:1])

            # ---- AllReduce #1: [preact(40) | S(1)] (hidden) -----------
            stage1 = cp.tile([128, 41], F32)
            nc.vector.tensor_copy(stage1[:, 0:40], psum_pre[:])
            nc.vector.tensor_copy(stage1[:, 40:41], rowsum[:])
            cc1_in = dp.tile([128, 41], F32)
            cc1_out = dp.tile([128, 41], F32, addr_space="Shared")
            nc.gpsimd.dma_start(cc1_in[:], stage1[:])
            nc.gpsimd.collective_compute(
                "AllReduce", OP.add,
                replica_groups=[list(range(N_CORES))],
                ins=[cc1_in[:]], outs=[cc1_out[:]])
            stage1o = cp.tile([128, 41], F32)
            nc.gpsimd.dma_start(stage1o[:], cc1_out[:])

            # ---- big matvec: p = e @ vals (fp32r, streamed) -----------
            p0 = pp.tile([1, 512], F32)
            p1 = pp.tile([1, 512], F32)
            g = 0
            for nb in BLOCKS:
                v = vp.tile([128, nb, H], F32R, tag="v")
                src = d["vals_s"][:, g * H:(g + nb) * H]
                nc.sync.dma_start(v[:], src.rearrange("p (c h) -> p c h", h=H))
                for c in range(nb):
                    e_col = e_r[:, g:g + 1]
                    nc.tensor.matmul(p0[:], e_col, v[:, c, 0:512],
                                     start=(g == 0), stop=(g == G - 1))
                    nc.tensor.matmul(p1[:], e_col, v[:, c, 512:1024],
                                     start=(g == 0), stop=(g == G - 1))
                    g += 1

            # ---- transpose p to [128, 8] ------------------------------
            p_sb = cp.tile([1, H], F32)
            nc.vector.tensor_copy(p_sb[0:1, 0:512], p0[:])
            nc.vector.tensor_copy(p_sb[0:1, 512:1024], p1[:])
            psum_mt = pp.tile([128, 8], F32)
            for n in range(8):
                nc.tensor.transpose(psum_mt[:, n:n + 1],
                                    p_sb[0:1, n * 128:(n + 1) * 128],
                                    ones_sb[0:1, 0:1])

            # ---- AllReduce #2: p (4 KB) -------------------------------
            stage2 = cp.tile([128, 8], F32)
            nc.vector.tensor_copy(stage2[:], psum_mt[:])
            cc2_in = dp.tile([128, 8], F32)
            cc2_out = dp.tile([128, 8], F32, addr_space="Shared")
            nc.gpsimd.dma_start(cc2_in[:], stage2[:])
            nc.gpsimd.collective_compute(
                "AllReduce", OP.add,
                replica_groups=[list(range(N_CORES))],
                ins=[cc2_in[:]], outs=[cc2_out[:]])
            # ---- gate math from AR1 (hidden under the vals stream) ----
            prefull = cp.tile([128, 40], F32)
            nc.vector.tensor_add(prefull[:], stage1o[:, 0:40], b5t_sb[:])
            th = cp.tile([128, 32], F32)
            nc.scalar.activation(th[:], prefull[:, 0:32], AF.Tanh, scale=0.5)
            gates = cp.tile([128, 32], F32)
            nc.vector.tensor_scalar(gates[:], th[:], 0.5, 0.5, OP.mult, OP.add)
            cnew = cp.tile([128, 8], F32)
            nc.scalar.activation(cnew[:], prefull[:, 32:40], AF.Tanh)
            psum_S = pp.tile([128, 1], F32, tag="ps_small")
            nc.tensor.matmul(psum_S[:], ones_sb[:], stage1o[:, 40:41])
            invS = cp.tile([128, 1], F32)
            nc.vector.reciprocal(invS[:], psum_S[:])
            t1 = cp.tile([128, 8], F32)
            nc.vector.tensor_mul(t1[:], gates[:, 0:8], c2t_sb[:])
            t2 = cp.tile([128, 8], F32)
            nc.vector.tensor_mul(t2[:], gates[:, 8:16], cnew[:])
            ct0 = cp.tile([128, 8], F32)
            nc.vector.tensor_add(ct0[:], t1[:], t2[:])

            stage2o = cp.tile([128, 8], F32)
            nc.gpsimd.dma_start(stage2o[:], cc2_out[:])

            # ---- LSTM tail --------------------------------------------
            mt_sb = cp.tile([128, 8], F32)
            nc.scalar.activation(mt_sb[:], stage2o[:], AF.Tanh,
                                 scale=invS[:, 0:1])
            t3 = cp.tile([128, 8], F32)
            nc.vector.tensor_mul(t3[:], gates[:, 24:32], mt_sb[:])
            ct = cp.tile([128, 8], F32)
            nc.vector.tensor_add(ct[:], ct0[:], t3[:])
            tct = cp.tile([128, 8], F32)
            nc.scalar.activation(tct[:], ct[:], AF.Tanh)
            ht = cp.tile([128, 8], F32)
            nc.vector.tensor_mul(ht[:], gates[:, 16:24], tct[:])
            ht_r = cp.tile([128, 8], BF16)
            nc.vector.tensor_copy(ht_r[:], ht[:])

            # ---- A2C head: hh = relu(W_ih @ h_t + b_ih) ---------------
            # moving-operand form: p0/p1 banks reused, 16 N=512 matmuls
            for c in range(8):
                nc.tensor.matmul(p0[:], ht_r[:, c:c + 1],
                                 wiht_sb[:, c, 0:512],
                                 start=(c == 0), stop=(c == 7))
                nc.tensor.matmul(p1[:], ht_r[:, c:c + 1],
                                 wiht_sb[:, c, 512:1024],
                                 start=(c == 0), stop=(c == 7))
            hh_row = cp.tile([1, H], F32)
            nc.vector.tensor_copy(hh_row[0:1, 0:512], p0[:])
            nc.vector.tensor_copy(hh_row[0:1, 512:1024], p1[:])
            for n in range(8):
                nc.tensor.transpose(psum_mt[:, n:n + 1],
                                    hh_row[0:1, n * 128:(n + 1) * 128],
                                    ones_sb[0:1, 0:1])
            hhb_sb = cp.tile([128, 8], F32)
            nc.vector.tensor_add(hhb_sb[:], psum_mt[:], biht_sb[:])
            hh_sb = cp.tile([128, 8], F32)
            nc.scalar.activation(hh_sb[:], hhb_sb[:], AF.Relu)

            psum_av = pp.tile([1, 3], F32, tag="pre0")
            for c in range(8):
                nc.tensor.matmul(psum_av[:], hh_sb[:, c:c + 1],
                                 wact_sb[:, c * 3:(c + 1) * 3],
                                 start=(c == 0), stop=(c == 7))
            av = cp.tile([1, 3], F32)
            nc.vector.tensor_add(av[:], psum_av[:], bac_sb[:])

            # ---- outputs ----------------------------------------------
            out_sb = cp.tile([128, 16], F32)
            nc.vector.tensor_copy(out_sb[:, 0:8], ht[:])
            nc.vector.tensor_copy(out_sb[:, 8:16], ct[:])
            nc.sync.dma_start(out_hc[:], out_sb[:])
            nc.sync.dma_start(out_av[:], av[:])

    nc.compile()
    return nc


def _get_nc():
    if "nc" not in _CACHE:
        _CACHE["nc"] = _build()
    return _CACHE["nc"]


def _prep_in_maps(x_t, h, c, keys, vals, W_i2h, b_i2h, W_h2h, b_h2h,
                  W_ih, b_ih, W_actor, b_actor, W_critic, b_critic, pick_arm):
    f = np.float32
    x_t = np.asarray(x_t, f)
    h = np.asarray(h, f).reshape(-1)          # [H]
    c = np.asarray(c, f).reshape(-1)          # [H]
    keys = np.asarray(keys, f)
    vals = np.asarray(vals, f)

    pa = int(np.asarray(pick_arm))
    start = min(max(pa * RD, 0), IN_DIM - RD)  # jax dynamic_slice clamping
    q = x_t[0, start:start + RD]

    q_rep = np.ascontiguousarray(
        np.broadcast_to(np.tile(q, G), (128, G * RD)))

    b5 = (np.asarray(b_i2h, f) + np.asarray(b_h2h, f))
    b5t = np.ascontiguousarray(b5.reshape(40, 128).T)
    biht = np.ascontiguousarray(np.asarray(b_ih, f).reshape(8, 128).T)
    c2t = np.ascontiguousarray(c.reshape(8, 128).T)

    BF = ml_dtypes.bfloat16
    wiht = np.ascontiguousarray(
        np.asarray(W_ih, f).T.reshape(8, 128, H).transpose(1, 0, 2)
        .reshape(128, 8 * H)).astype(BF)
    wac = np.vstack([np.asarray(W_actor, f), np.asarray(W_critic, f)])  # [3,H]
    wact = np.ascontiguousarray(
        wac.T.reshape(8, 128, 3).transpose(1, 0, 2).reshape(128, 24))
    bac = np.concatenate([np.asarray(b_actor, f),
                          np.asarray(b_critic, f)]).reshape(1, 3)

    W_i2hT = np.ascontiguousarray(np.asarray(W_i2h, f).T)
    wxt_zero = np.zeros_like(W_i2hT)
    x_col = np.ascontiguousarray(x_t[0].reshape(IN_DIM, 1))
    x_zero = np.zeros_like(x_col)

    in_maps = []
    for k in range(N_CORES):
        r0 = k * PER
        r1 = min(r0 + PER, D)
        n_valid = r1 - r0

        vals_p = np.zeros((PER, H), f)
        vals_p[:n_valid] = vals[r0:r1]
        vals_s = np.ascontiguousarray(
            vals_p.reshape(G, 128, H).transpose(1, 0, 2).reshape(128, G * H))
        keys_p = np.zeros((PER, RD), f)
        keys_p[:n_valid] = keys[r0:r1]
        keys_t = np.ascontiguousarray(
            keys_p.reshape(G, 128, RD).transpose(1, 0, 2).reshape(128, G * RD))
        idx = np.arange(G)[None, :] * 128 + np.arange(128)[:, None]
        mask = (idx < n_valid).astype(f)

        wht = np.ascontiguousarray(
            np.asarray(W_h2h, f)[:, k * 128:(k + 1) * 128].T)
        h_col = np.ascontiguousarray(h[k * 128:(k + 1) * 128].reshape(128, 1))

        in_maps.append({
            "vals_s": vals_s,
            "keys_t": keys_t,
            "q_rep": q_rep,
            "mask": mask,
            "wht": wht,
            "wxt": W_i2hT if k == 0 else wxt_zero,
            "x_col": x_col if k == 0 else x_zero,
            "h_col": h_col,
            "c2t": c2t,
            "b5t": b5t,
            "biht": biht,
            "wiht": wiht,
            "wact": wact,
            "bac": bac,
        })
    return in_maps


def _postprocess(out_hc, out_av):
    h_t = np.ascontiguousarray(out_hc[:, 0:8].T).reshape(-1)
    c_t = np.ascontiguousarray(out_hc[:, 8:16].T).reshape(-1)
    logits = out_av[0, 0:2].astype(np.float32)
    v = np.float32(out_av[0, 2])
    m = logits.max()
    ex = np.exp(logits - m)
    pi = (ex / ex.sum()).astype(np.float32)
    a = int(np.argmax(np.log(pi) + GUMBEL))
    logp = np.float32(np.log(pi[a]))
    return np.concatenate([pi, [v], [logp], h_t, c_t]).astype(np.float32)


def kernel(**inputs) -> np.ndarray:
    nc = _get_nc()
    in_maps = _prep_in_maps(**inputs)
    res = run_bass_kernel_spmd(
        nc, in_maps, core_ids=list(range(N_CORES)),
        **_CACHE.get("run_kwargs", {}))
    _CACHE["last_results"] = res
    r0 = res.results[0]
    return _postprocess(r0["out_hc"], r0["out_av"])


# revision 13
# speedup vs baseline: 1.0315x; 1.0315x over previous
"""Trainium2 Bass kernel for CompositionalTwoArmedAgent (DND-LSTM A2C step).

Strategy (8 NeuronCores, SPMD + AllReduce):
  - DND keys/vals tables sharded row-wise: 12544 rows/core (core 7 padded).
  - Cosine similarities are bounded in [-1, 1], so the softmax needs no
    max pass: each core computes e_i = exp(cos_i), a partial sum S_k and a
    partial weighted value sum p_k = e @ vals_k (TensorE, fp32r full rate).
  - The LSTM i2h/h2h GEMM is sharded over its contraction dim (128 h-dims
    per core; the x_t @ W_i2h.T part is zero-fed on cores 1..7).
  - Two AllReduces: [preact(5120) | S(1)] early (hidden under the vals
    stream, so the gate math is precomputed), p(1024) late (4 KB).
  - Every core then computes the identical tiny LSTM/A2C tail; host reads
    core 0's output, applies the 2-class softmax / fixed-key categorical
    sample, and packs the reference's output layout.
"""

import ml_dtypes
import numpy as np

import concourse.bacc as bacc
import concourse.bass as bass
import concourse.mybir as mybir
import concourse.tile as tile
from concourse.bass_utils import run_bass_kernel_spmd

N_CORES = 8
D, RD, H, IN_DIM, A = 100000, 10, 1024, 14, 2
PER = 12544            # padded rows per core = G * 128
G = 98                 # 128-row chunks per core
BLOCKS = [7] * 13 + [4, 2, 1]   # chunks per vals DMA block (descending tail)
F32 = mybir.dt.float32
F32R = mybir.dt.float32r
BF16 = mybir.dt.bfloat16

# jax.random.gumbel(jax.random.key(1), (2,), float32) — fixed constants of the
# reference's categorical sample (verified against jax.random.categorical).
GUMBEL = np.array([0.5325072, -0.01641824], np.float32)

_CACHE = {}


def _input_specs():
    return [
        ("vals_s", [128, G * H], F32R),    # row-chunk-tiled vals shard
        ("keys_t", [128, G * RD], F32),
        ("q_rep", [128, G * RD], F32),
        ("mask", [128, G], F32),
        ("wht", [128, 5 * H], F32R),
        ("wxt", [IN_DIM, 5 * H], F32R),
        ("x_col", [IN_DIM, 1], F32R),
        ("h_col", [128, 1], F32R),
        ("c2t", [128, 8], F32),
        ("b5t", [128, 40], F32),
        ("biht", [128, 8], F32),
        ("wiht", [128, 8 * H], BF16),
        ("wact", [128, 24], F32),
        ("bac", [1, 3], F32),
    ]


def _build():
    nc = bacc.Bacc("TRN2", target_bir_lowering=False, debug=False,
                   num_devices=N_CORES)
    d = {name: nc.dram_tensor(name, shp, dt, kind="ExternalInput")
         for name, shp, dt in _input_specs()}
    out_hc = nc.dram_tensor("out_hc", [128, 16], F32, kind="ExternalOutput")
    out_av = nc.dram_tensor("out_av", [1, 3], F32, kind="ExternalOutput")

    AF = mybir.ActivationFunctionType
    OP = mybir.AluOpType

    with tile.TileContext(nc) as tc:
        with (
            tc.tile_pool(name="const", bufs=1) as cp,
            tc.tile_pool(name="vals", bufs=3) as vp,
            tc.tile_pool(name="ps", bufs=1, space="PSUM") as pp,
            tc.tile_pool(name="dram", bufs=1, space="DRAM") as dp,
        ):
            # ---- persistent loads -------------------------------------
            keys_sb = cp.tile([128, G * RD], F32)
            q_sb = cp.tile([128, G * RD], F32)
            mask_sb = cp.tile([128, G], F32)
            wht_sb = cp.tile([128, 5 * H], F32R)
            wxt_sb = cp.tile([IN_DIM, 5 * H], F32R)
            x_col_sb = cp.tile([IN_DIM, 1], F32R)
            h_col_sb = cp.tile([128, 1], F32R)
            c2t_sb = cp.tile([128, 8], F32)
            b5t_sb = cp.tile([128, 40], F32)
            biht_sb = cp.tile([128, 8], F32)
            wiht_sb = cp.tile([128, 8, H], BF16)
            wact_sb = cp.tile([128, 24], F32)
            bac_sb = cp.tile([1, 3], F32)
            for name, t in [("keys_t", keys_sb), ("q_rep", q_sb),
                            ("mask", mask_sb), ("wht", wht_sb),
                            ("wxt", wxt_sb), ("x_col", x_col_sb),
                            ("h_col", h_col_sb), ("c2t", c2t_sb),
                            ("b5t", b5t_sb), ("biht", biht_sb),
                            ("wact", wact_sb), ("bac", bac_sb)]:
                nc.scalar.dma_start(t[:], d[name][:])
            nc.scalar.dma_start(
                wiht_sb[:], d["wiht"][:].rearrange("p (c j) -> p c j", j=H))

            ones_sb = cp.tile([128, 128], F32)
            nc.vector.memset(ones_sb[:], 1.0)

            # ---- ||q||^2 broadcast to all partitions ------------------
            sq_q = cp.tile([1, RD], F32)
            nc.scalar.activation(sq_q[:], q_sb[0:1, 0:RD], AF.Square)
            qnsq = cp.tile([1, 1], F32)
            nc.vector.reduce_sum(qnsq[:], sq_q[:], axis=mybir.AxisListType.X)
            psum_qn = pp.tile([128, 1], F32, tag="ps_small")
            nc.tensor.matmul(psum_qn[:], ones_sb[0:1, :], qnsq[:])
            qn2b = cp.tile([128, 1], F32)
            nc.vector.tensor_copy(qn2b[:], psum_qn[:])

            # ---- cosine sims -> masked exp weights --------------------
            prod = cp.tile([128, G * RD], F32)
            nc.vector.tensor_mul(prod[:], keys_sb[:], q_sb[:])
            dots = cp.tile([128, G], F32)
            nc.vector.tensor_reduce(
                dots[:], prod[:].rearrange("p (g r) -> p g r", r=RD),
                axis=mybir.AxisListType.X, op=OP.add)
            sqk = cp.tile([128, G * RD], F32)
            nc.scalar.activation(sqk[:], keys_sb[:], AF.Square)
            nsq = cp.tile([128, G], F32)
            nc.vector.tensor_reduce(
                nsq[:], sqk[:].rearrange("p (g r) -> p g r", r=RD),
                axis=mybir.AxisListType.X, op=OP.add)
            d2 = cp.tile([128, G], F32)
            nc.vector.tensor_scalar(d2[:], nsq[:], qn2b[:, 0:1], None, OP.mult)
            den = cp.tile([128, G], F32)
            nc.scalar.activation(den[:], d2[:], AF.Sqrt)
            denc = cp.tile([128, G], F32)
            nc.vector.tensor_scalar_max(denc[:], den[:], 1e-8)
            rec = cp.tile([128, G], F32)
            nc.vector.reciprocal(rec[:], denc[:])
            s_sb = cp.tile([128, G], F32)
            nc.vector.tensor_mul(s_sb[:], dots[:], rec[:])
            eraw = cp.tile([128, G], F32)
            nc.scalar.activation(eraw[:], s_sb[:], AF.Exp)
            e_sb = cp.tile([128, G], F32)
            rowsum = cp.tile([128, 1], F32)
            nc.vector.scalar_tensor_tensor(
                e_sb[:], eraw[:], 1.0, mask_sb[:], OP.mult, OP.mult,
                accum_out=rowsum[:])
            e_r = cp.tile([128, G], F32R)
            nc.vector.tensor_copy(e_r[:], e_sb[:])

            # ---- preact partial: [x;h_chunk] @ [WxT;WhT] --------------
            # moving-operand fp32r form: 20 N=512 matmuls into [1,512] rows,
            # then 40 PE transposes back to the compact [128, 40] col layout.
            psum_pre = pp.tile([128, 40], F32)
            for n in range(10):
                pre_ps = pp.tile([1, 512], F32, tag=f"pre{n % 2}")
                nc.tensor.matmul(pre_ps[:], h_col_sb[:],
                                 wht_sb[:, n * 512:(n + 1) * 512],
                                 start=True, stop=False)
                nc.tensor.matmul(pre_ps[:], x_col_sb[:],
                                 wxt_sb[:, n * 512:(n + 1) * 512],
                                 start=False, stop=True)
                row_scr = cp.tile([1, 512], F32, tag="rowscr", bufs=2)
                nc.vector.tensor_copy(row_scr[:], pre_ps[:])
                for t in range(4):
                    nc.tensor.transpose(psum_pre[:, 4 * n + t:4 * n + t + 1],
                                        row_scr[0:1, t * 128:(t + 1) * 128],
                                        ones_sb[0:1, 0:1])

            # ---- AllReduce #1: [preact(40) | S(1)] (hidden) -----------
            stage1 = cp.tile([128, 41], F32)
            nc.vector.tensor_copy(stage1[:, 0:40], psum_pre[:])
            nc.vector.tensor_copy(stage1[:, 40:41], rowsum[:])
            cc1_in = dp.tile([128, 41], F32)
            cc1_out = dp.tile([128, 41], F32, addr_space="Shared")
            nc.gpsimd.dma_start(cc1_in[:], stage1[:])
            nc.gpsimd.collective_compute(
                "AllReduce", OP.add,
                replica_groups=[list(range(N_CORES))],
                ins=[cc1_in[:]], outs=[cc1_out[:]])
            stage1o = cp.tile([128, 41], F32)
            nc.gpsimd.dma_start(stage1o[:], cc1_out[:])

            # ---- big matvec: p = e @ vals (fp32r, streamed) -----------
            p0 = pp.tile([1, 512], F32)
            p1 = pp.tile([1, 512], F32)
            g = 0
            for nb in BLOCKS:
                v = vp.tile([128, nb, H], F32R, tag="v")
                src = d["vals_s"][:, g * H:(g + nb) * H]
                nc.sync.dma_start(v[:], src.rearrange("p (c h) -> p c h", h=H))
                for c in range(nb):
                    e_col = e_r[:, g:g + 1]
                    nc.tensor.matmul(p0[:], e_col, v[:, c, 0:512],
                                     start=(g == 0), stop=(g == G - 1))
                    nc.tensor.matmul(p1[:], e_col, v[:, c, 512:1024],
                                     start=(g == 0), stop=(g == G - 1))
                    g += 1

            # ---- transpose p to [128, 8] ------------------------------
            p_sb = cp.tile([1, H], F32)
            nc.vector.tensor_copy(p_sb[0:1, 0:512], p0[:])
            nc.vector.tensor_copy(p_sb[0:1, 512:1024], p1[:])
            psum_mt = pp.tile([128, 8], F32)
            for n in range(8):
                nc.tensor.transpose(psum_mt[:, n:n + 1],
                                    p_sb[0:1, n * 128:(n + 1) * 128],
                                    ones_sb[0:1, 0:1])

            # ---- AllReduce #2: p (4 KB) -------------------------------
            stage2 = cp.tile([128, 8], F32)
            nc.vector.tensor_copy(stage2[:], psum_mt[:])
            cc2_in = dp.tile([128, 8], F32)
            cc2_out = dp.tile([128, 8], F32, addr_space="Shared")
            nc.gpsimd.dma_start(cc2_in[:], stage2[:])
            nc.gpsimd.collective_compute(
                "AllReduce", OP.add,
                replica_groups=[list(range(N_CORES))],
                ins=[cc2_in[:]], outs=[cc2_out[:]])
            # ---- gate math from AR1 (hidden under the vals stream) ----
            prefull = cp.tile([128, 40], F32)
            nc.vector.tensor_add(prefull[:], stage1o[:, 0:40], b5t_sb[:])
            th = cp.tile([128, 32], F32)
            nc.scalar.activation(th[:], prefull[:, 0:32], AF.Tanh, scale=0.5)
            gates = cp.tile([128, 32], F32)
            nc.vector.tensor_scalar(gates[:], th[:], 0.5, 0.5, OP.mult, OP.add)
            cnew = cp.tile([128, 8], F32)
            nc.scalar.activation(cnew[:], prefull[:, 32:40], AF.Tanh)
            S_all = cp.tile([128, 1], F32)
            nc.gpsimd.partition_all_reduce(
                S_all[:], stage1o[:, 40:41], 128,
                bass.bass_isa.ReduceOp.add)
            invS = cp.tile([128, 1], F32)
            nc.vector.reciprocal(invS[:], S_all[:])
            t1 = cp.tile([128, 8], F32)
            nc.vector.tensor_mul(t1[:], gates[:, 0:8], c2t_sb[:])
            t2 = cp.tile([128, 8], F32)
            nc.vector.tensor_mul(t2[:], gates[:, 8:16], cnew[:])
            ct0 = cp.tile([128, 8], F32)
            nc.vector.tensor_add(ct0[:], t1[:], t2[:])

            stage2o = cp.tile([128, 8], F32)
            nc.gpsimd.dma_start(stage2o[:], cc2_out[:])

            # ---- LSTM tail --------------------------------------------
            mt_sb = cp.tile([128, 8], F32)
            nc.scalar.activation(mt_sb[:], stage2o[:], AF.Tanh,
                                 scale=invS[:, 0:1])
            t3 = cp.tile([128, 8], F32)
            nc.vector.tensor_mul(t3[:], gates[:, 24:32], mt_sb[:])
            ct = cp.tile([128, 8], F32)
            nc.vector.tensor_add(ct[:], ct0[:], t3[:])
            tct = cp.tile([128, 8], F32)
            nc.scalar.activation(tct[:], ct[:], AF.Tanh)
            ht = cp.tile([128, 8], F32)
            nc.vector.tensor_mul(ht[:], gates[:, 16:24], tct[:])
            ht_r = cp.tile([128, 8], BF16)
            nc.vector.tensor_copy(ht_r[:], ht[:])

            # ---- A2C head: hh = relu(W_ih @ h_t + b_ih) ---------------
            # moving-operand form: p0/p1 banks reused, 16 N=512 matmuls
            for c in range(8):
                nc.tensor.matmul(p0[:], ht_r[:, c:c + 1],
                                 wiht_sb[:, c, 0:512],
                                 start=(c == 0), stop=(c == 7))
                nc.tensor.matmul(p1[:], ht_r[:, c:c + 1],
                                 wiht_sb[:, c, 512:1024],
                                 start=(c == 0), stop=(c == 7))
            hh_row = cp.tile([1, H], F32)
            nc.vector.tensor_copy(hh_row[0:1, 0:512], p0[:])
            nc.vector.tensor_copy(hh_row[0:1, 512:1024], p1[:])
            for n in range(8):
                nc.tensor.transpose(psum_mt[:, n:n + 1],
                                    hh_row[0:1, n * 128:(n + 1) * 128],
                                    ones_sb[0:1, 0:1])
            hhb_sb = cp.tile([128, 8], F32)
            nc.vector.tensor_add(hhb_sb[:], psum_mt[:], biht_sb[:])
            hh_sb = cp.tile([128, 8], F32)
            nc.scalar.activation(hh_sb[:], hhb_sb[:], AF.Relu)

            psum_av = pp.tile([1, 3], F32, tag="pre0")
            for c in range(8):
                nc.tensor.matmul(psum_av[:], hh_sb[:, c:c + 1],
                                 wact_sb[:, c * 3:(c + 1) * 3],
                                 start=(c == 0), stop=(c == 7))
            av = cp.tile([1, 3], F32)
            nc.vector.tensor_add(av[:], psum_av[:], bac_sb[:])

            # ---- outputs ----------------------------------------------
            out_sb = cp.tile([128, 16], F32)
            nc.vector.tensor_copy(out_sb[:, 0:8], ht[:])
            nc.vector.tensor_copy(out_sb[:, 8:16], ct[:])
            nc.sync.dma_start(out_hc[:], out_sb[:])
            nc.sync.dma_start(out_av[:], av[:])

    nc.compile()
    return nc


def _get_nc():
    if "nc" not in _CACHE:
        _CACHE["nc"] = _build()
    return _CACHE["nc"]


def _prep_in_maps(x_t, h, c, keys, vals, W_i2h, b_i2h, W_h2h, b_h2h,
                  W_ih, b_ih, W_actor, b_actor, W_critic, b_critic, pick_arm):
    f = np.float32
    x_t = np.asarray(x_t, f)
    h = np.asarray(h, f).reshape(-1)          # [H]
    c = np.asarray(c, f).reshape(-1)          # [H]
    keys = np.asarray(keys, f)
    vals = np.asarray(vals, f)

    pa = int(np.asarray(pick_arm))
    start = min(max(pa * RD, 0), IN_DIM - RD)  # jax dynamic_slice clamping
    q = x_t[0, start:start + RD]

    q_rep = np.ascontiguousarray(
        np.broadcast_to(np.tile(q, G), (128, G * RD)))

    b5 = (np.asarray(b_i2h, f) + np.asarray(b_h2h, f))
    b5t = np.ascontiguousarray(b5.reshape(40, 128).T)
    biht = np.ascontiguousarray(np.asarray(b_ih, f).reshape(8, 128).T)
    c2t = np.ascontiguousarray(c.reshape(8, 128).T)

    BF = ml_dtypes.bfloat16
    wiht = np.ascontiguousarray(
        np.asarray(W_ih, f).T.reshape(8, 128, H).transpose(1, 0, 2)
        .reshape(128, 8 * H)).astype(BF)
    wac = np.vstack([np.asarray(W_actor, f), np.asarray(W_critic, f)])  # [3,H]
    wact = np.ascontiguousarray(
        wac.T.reshape(8, 128, 3).transpose(1, 0, 2).reshape(128, 24))
    bac = np.concatenate([np.asarray(b_actor, f),
                          np.asarray(b_critic, f)]).reshape(1, 3)

    W_i2hT = np.ascontiguousarray(np.asarray(W_i2h, f).T)
    wxt_zero = np.zeros_like(W_i2hT)
    x_col = np.ascontiguousarray(x_t[0].reshape(IN_DIM, 1))
    x_zero = np.zeros_like(x_col)

    in_maps = []
    for k in range(N_CORES):
        r0 = k * PER
        r1 = min(r0 + PER, D)
        n_valid = r1 - r0

        vals_p = np.zeros((PER, H), f)
        vals_p[:n_valid] = vals[r0:r1]
        vals_s = np.ascontiguousarray(
            vals_p.reshape(G, 128, H).transpose(1, 0, 2).reshape(128, G * H))
        keys_p = np.zeros((PER, RD), f)
        keys_p[:n_valid] = keys[r0:r1]
        keys_t = np.ascontiguousarray(
            keys_p.reshape(G, 128, RD).transpose(1, 0, 2).reshape(128, G * RD))
        idx = np.arange(G)[None, :] * 128 + np.arange(128)[:, None]
        mask = (idx < n_valid).astype(f)

        wht = np.ascontiguousarray(
            np.asarray(W_h2h, f)[:, k * 128:(k + 1) * 128].T)
        h_col = np.ascontiguousarray(h[k * 128:(k + 1) * 128].reshape(128, 1))

        in_maps.append({
            "vals_s": vals_s,
            "keys_t": keys_t,
            "q_rep": q_rep,
            "mask": mask,
            "wht": wht,
            "wxt": W_i2hT if k == 0 else wxt_zero,
            "x_col": x_col if k == 0 else x_zero,
            "h_col": h_col,
            "c2t": c2t,
            "b5t": b5t,
            "biht": biht,
            "wiht": wiht,
            "wact": wact,
            "bac": bac,
        })
    return in_maps


def _postprocess(out_hc, out_av):
    h_t = np.ascontiguousarray(out_hc[:, 0:8].T).reshape(-1)
    c_t = np.ascontiguousarray(out_hc[:, 8:16].T).reshape(-1)
    logits = out_av[0, 0:2].astype(np.float32)
    v = np.float32(out_av[0, 2])
    m = logits.max()
    ex = np.exp(logits - m)
    pi = (ex / ex.sum()).astype(np.float32)
    a = int(np.argmax(np.log(pi) + GUMBEL))
    logp = np.float32(np.log(pi[a]))
    return np.concatenate([pi, [v], [logp], h_t, c_t]).astype(np.float32)


def kernel(**inputs) -> np.ndarray:
    nc = _get_nc()
    in_maps = _prep_in_maps(**inputs)
    res = run_bass_kernel_spmd(
        nc, in_maps, core_ids=list(range(N_CORES)),
        **_CACHE.get("run_kwargs", {}))
    _CACHE["last_results"] = res
    r0 = res.results[0]
    return _postprocess(r0["out_hc"], r0["out_av"])


# revision 14
# speedup vs baseline: 1.1249x; 1.0905x over previous
"""Trainium2 Bass kernel for CompositionalTwoArmedAgent (DND-LSTM A2C step).

Strategy (8 NeuronCores, SPMD + AllReduce):
  - DND keys/vals tables sharded row-wise: 12544 rows/core (core 7 padded).
  - Cosine similarities are bounded in [-1, 1], so the softmax needs no
    max pass: each core computes e_i = exp(cos_i), a partial sum S_k and a
    partial weighted value sum p_k = e @ vals_k (TensorE, fp32r full rate).
  - The LSTM i2h/h2h GEMM is sharded over its contraction dim (128 h-dims
    per core; the x_t @ W_i2h.T part is zero-fed on cores 1..7).
  - Two AllReduces: [preact(5120) | S(1)] early (hidden under the vals
    stream, so the gate math is precomputed), p(1024) late (4 KB).
  - Every core then computes the identical tiny LSTM/A2C tail; host reads
    core 0's output, applies the 2-class softmax / fixed-key categorical
    sample, and packs the reference's output layout.
"""

import ml_dtypes
import numpy as np

import concourse.bacc as bacc
import concourse.bass as bass
import concourse.mybir as mybir
import concourse.tile as tile
from concourse.bass_utils import run_bass_kernel_spmd

N_CORES = 8
D, RD, H, IN_DIM, A = 100000, 10, 1024, 14, 2
PER = 12544            # padded rows per core = G * 128
G = 98                 # 128-row chunks per core
BLOCKS = [7] * 13 + [4, 2, 1]   # chunks per vals DMA block (descending tail)
F32 = mybir.dt.float32
F32R = mybir.dt.float32r
BF16 = mybir.dt.bfloat16

# jax.random.gumbel(jax.random.key(1), (2,), float32) — fixed constants of the
# reference's categorical sample (verified against jax.random.categorical).
GUMBEL = np.array([0.5325072, -0.01641824], np.float32)

_CACHE = {}


def _input_specs():
    return [
        ("vals_s", [128, G * H], F32R),    # row-chunk-tiled vals shard
        ("keys_t", [128, G * RD], F32),
        ("q_rep", [128, G * RD], F32),
        ("mask", [128, G], F32),
        ("wht", [128, 5 * H], F32R),
        ("wxt", [IN_DIM, 5 * H], F32R),
        ("x_col", [IN_DIM, 1], F32R),
        ("h_col", [128, 1], F32R),
        ("c2t", [128, 8], F32),
        ("b5t", [128, 40], F32),
        ("biht", [128, 8], F32),
        ("wiht", [128, 8 * H], BF16),
        ("wact", [128, 24], F32),
        ("bac", [1, 3], F32),
    ]


def _build():
    nc = bacc.Bacc("TRN2", target_bir_lowering=False, debug=False,
                   num_devices=N_CORES)
    d = {name: nc.dram_tensor(name, shp, dt, kind="ExternalInput")
         for name, shp, dt in _input_specs()}
    out_hc = nc.dram_tensor("out_hc", [128, 16], F32, kind="ExternalOutput")
    out_av = nc.dram_tensor("out_av", [1, 3], F32, kind="ExternalOutput")

    AF = mybir.ActivationFunctionType
    OP = mybir.AluOpType

    with tile.TileContext(nc) as tc:
        with (
            tc.tile_pool(name="const", bufs=1) as cp,
            tc.tile_pool(name="vals", bufs=3) as vp,
            tc.tile_pool(name="ps", bufs=1, space="PSUM") as pp,
            tc.tile_pool(name="dram", bufs=1, space="DRAM") as dp,
        ):
            # ---- persistent loads -------------------------------------
            keys_sb = cp.tile([128, G * RD], F32)
            q_sb = cp.tile([128, G * RD], F32)
            mask_sb = cp.tile([128, G], F32)
            wht_sb = cp.tile([128, 5 * H], F32R)
            wxt_sb = cp.tile([IN_DIM, 5 * H], F32R)
            x_col_sb = cp.tile([IN_DIM, 1], F32R)
            h_col_sb = cp.tile([128, 1], F32R)
            c2t_sb = cp.tile([128, 8], F32)
            b5t_sb = cp.tile([128, 40], F32)
            biht_sb = cp.tile([128, 8], F32)
            wiht_sb = cp.tile([128, 8, H], BF16)
            wact_sb = cp.tile([128, 24], F32)
            bac_sb = cp.tile([1, 3], F32)
            for name, t in [("wht", wht_sb), ("wxt", wxt_sb),
                            ("x_col", x_col_sb), ("h_col", h_col_sb),
                            ("keys_t", keys_sb), ("q_rep", q_sb),
                            ("mask", mask_sb), ("c2t", c2t_sb),
                            ("b5t", b5t_sb), ("biht", biht_sb),
                            ("wact", wact_sb), ("bac", bac_sb)]:
                nc.scalar.dma_start(t[:], d[name][:])
            nc.scalar.dma_start(
                wiht_sb[:], d["wiht"][:].rearrange("p (c j) -> p c j", j=H))

            ones_sb = cp.tile([128, 128], F32)
            nc.vector.memset(ones_sb[:], 1.0)

            # ---- ||q||^2 broadcast to all partitions ------------------
            sq_q = cp.tile([1, RD], F32)
            nc.scalar.activation(sq_q[:], q_sb[0:1, 0:RD], AF.Square)
            qnsq = cp.tile([1, 1], F32)
            nc.vector.reduce_sum(qnsq[:], sq_q[:], axis=mybir.AxisListType.X)
            psum_qn = pp.tile([128, 1], F32, tag="ps_small")
            nc.tensor.matmul(psum_qn[:], ones_sb[0:1, :], qnsq[:])
            qn2b = cp.tile([128, 1], F32)
            nc.vector.tensor_copy(qn2b[:], psum_qn[:])

            # ---- cosine sims -> masked exp weights --------------------
            prod = cp.tile([128, G * RD], F32)
            nc.vector.tensor_mul(prod[:], keys_sb[:], q_sb[:])
            dots = cp.tile([128, G], F32)
            nc.vector.tensor_reduce(
                dots[:], prod[:].rearrange("p (g r) -> p g r", r=RD),
                axis=mybir.AxisListType.X, op=OP.add)
            sqk = cp.tile([128, G * RD], F32)
            nc.scalar.activation(sqk[:], keys_sb[:], AF.Square)
            nsq = cp.tile([128, G], F32)
            nc.vector.tensor_reduce(
                nsq[:], sqk[:].rearrange("p (g r) -> p g r", r=RD),
                axis=mybir.AxisListType.X, op=OP.add)
            d2 = cp.tile([128, G], F32)
            nc.vector.tensor_scalar(d2[:], nsq[:], qn2b[:, 0:1], None, OP.mult)
            den = cp.tile([128, G], F32)
            nc.scalar.activation(den[:], d2[:], AF.Sqrt)
            denc = cp.tile([128, G], F32)
            nc.vector.tensor_scalar_max(denc[:], den[:], 1e-8)
            rec = cp.tile([128, G], F32)
            nc.vector.reciprocal(rec[:], denc[:])
            s_sb = cp.tile([128, G], F32)
            nc.vector.tensor_mul(s_sb[:], dots[:], rec[:])
            eraw = cp.tile([128, G], F32)
            nc.scalar.activation(eraw[:], s_sb[:], AF.Exp)
            e_sb = cp.tile([128, G], F32)
            rowsum = cp.tile([128, 1], F32)
            nc.vector.scalar_tensor_tensor(
                e_sb[:], eraw[:], 1.0, mask_sb[:], OP.mult, OP.mult,
                accum_out=rowsum[:])
            e_r = cp.tile([128, G], F32R)
            nc.vector.tensor_copy(e_r[:], e_sb[:])

            # ---- preact partial: [x;h_chunk] @ [WxT;WhT] --------------
            # moving-operand fp32r form: 20 N=512 matmuls into [1,512] rows,
            # then 40 PE transposes back to the compact [128, 40] col layout.
            psum_pre = pp.tile([128, 40], F32)
            for n in range(10):
                pre_ps = pp.tile([1, 512], F32, tag=f"pre{n % 2}")
                nc.tensor.matmul(pre_ps[:], h_col_sb[:],
                                 wht_sb[:, n * 512:(n + 1) * 512],
                                 start=True, stop=False)
                nc.tensor.matmul(pre_ps[:], x_col_sb[:],
                                 wxt_sb[:, n * 512:(n + 1) * 512],
                                 start=False, stop=True)
                row_scr = cp.tile([1, 512], F32, tag="rowscr", bufs=2)
                nc.vector.tensor_copy(row_scr[:], pre_ps[:])
                for t in range(4):
                    nc.tensor.transpose(psum_pre[:, 4 * n + t:4 * n + t + 1],
                                        row_scr[0:1, t * 128:(t + 1) * 128],
                                        ones_sb[0:1, 0:1])

            # ---- AllReduce #1: [preact(40) | S(1)] (hidden) -----------
            stage1 = cp.tile([128, 41], F32)
            nc.vector.tensor_copy(stage1[:, 0:40], psum_pre[:])
            nc.vector.tensor_copy(stage1[:, 40:41], rowsum[:])
            cc1_in = dp.tile([128, 41], F32)
            cc1_out = dp.tile([128, 41], F32, addr_space="Shared")
            nc.gpsimd.dma_start(cc1_in[:], stage1[:])
            nc.gpsimd.collective_compute(
                "AllReduce", OP.add,
                replica_groups=[list(range(N_CORES))],
                ins=[cc1_in[:]], outs=[cc1_out[:]])
            stage1o = cp.tile([128, 41], F32)
            nc.gpsimd.dma_start(stage1o[:], cc1_out[:])

            # ---- big matvec: p = e @ vals (fp32r, streamed) -----------
            p0 = pp.tile([1, 512], F32)
            p1 = pp.tile([1, 512], F32)
            g = 0
            for nb in BLOCKS:
                v = vp.tile([128, nb, H], F32R, tag="v")
                src = d["vals_s"][:, g * H:(g + nb) * H]
                nc.sync.dma_start(v[:], src.rearrange("p (c h) -> p c h", h=H))
                for c in range(nb):
                    e_col = e_r[:, g:g + 1]
                    nc.tensor.matmul(p0[:], e_col, v[:, c, 0:512],
                                     start=(g == 0), stop=(g == G - 1))
                    nc.tensor.matmul(p1[:], e_col, v[:, c, 512:1024],
                                     start=(g == 0), stop=(g == G - 1))
                    g += 1

            # ---- transpose p to [128, 8] ------------------------------
            p_sb = cp.tile([1, H], F32)
            nc.vector.tensor_copy(p_sb[0:1, 0:512], p0[:])
            nc.vector.tensor_copy(p_sb[0:1, 512:1024], p1[:])
            psum_mt = pp.tile([128, 8], F32)
            for n in range(8):
                nc.tensor.transpose(psum_mt[:, n:n + 1],
                                    p_sb[0:1, n * 128:(n + 1) * 128],
                                    ones_sb[0:1, 0:1])

            # ---- AllReduce #2: p (4 KB) -------------------------------
            stage2 = cp.tile([128, 8], F32)
            nc.vector.tensor_copy(stage2[:], psum_mt[:])
            cc2_in = dp.tile([128, 8], F32)
            cc2_out = dp.tile([128, 8], F32, addr_space="Shared")
            nc.gpsimd.dma_start(cc2_in[:], stage2[:])
            nc.gpsimd.collective_compute(
                "AllReduce", OP.add,
                replica_groups=[list(range(N_CORES))],
                ins=[cc2_in[:]], outs=[cc2_out[:]])
            # ---- gate math from AR1 (hidden under the vals stream) ----
            prefull = cp.tile([128, 40], F32)
            nc.vector.tensor_add(prefull[:], stage1o[:, 0:40], b5t_sb[:])
            th = cp.tile([128, 32], F32)
            nc.scalar.activation(th[:], prefull[:, 0:32], AF.Tanh, scale=0.5)
            gates = cp.tile([128, 32], F32)
            nc.vector.tensor_scalar(gates[:], th[:], 0.5, 0.5, OP.mult, OP.add)
            cnew = cp.tile([128, 8], F32)
            nc.scalar.activation(cnew[:], prefull[:, 32:40], AF.Tanh)
            S_all = cp.tile([128, 1], F32)
            nc.gpsimd.partition_all_reduce(
                S_all[:], stage1o[:, 40:41], 128,
                bass.bass_isa.ReduceOp.add)
            invS = cp.tile([128, 1], F32)
            nc.vector.reciprocal(invS[:], S_all[:])
            t1 = cp.tile([128, 8], F32)
            nc.vector.tensor_mul(t1[:], gates[:, 0:8], c2t_sb[:])
            t2 = cp.tile([128, 8], F32)
            nc.vector.tensor_mul(t2[:], gates[:, 8:16], cnew[:])
            ct0 = cp.tile([128, 8], F32)
            nc.vector.tensor_add(ct0[:], t1[:], t2[:])

            stage2o = cp.tile([128, 8], F32)
            nc.gpsimd.dma_start(stage2o[:], cc2_out[:])

            # ---- LSTM tail --------------------------------------------
            mt_sb = cp.tile([128, 8], F32)
            nc.scalar.activation(mt_sb[:], stage2o[:], AF.Tanh,
                                 scale=invS[:, 0:1])
            t3 = cp.tile([128, 8], F32)
            nc.vector.tensor_mul(t3[:], gates[:, 24:32], mt_sb[:])
            ct = cp.tile([128, 8], F32)
            nc.vector.tensor_add(ct[:], ct0[:], t3[:])
            tct = cp.tile([128, 8], F32)
            nc.scalar.activation(tct[:], ct[:], AF.Tanh)
            ht = cp.tile([128, 8], F32)
            nc.vector.tensor_mul(ht[:], gates[:, 16:24], tct[:])
            ht_r = cp.tile([128, 8], BF16)
            nc.vector.tensor_copy(ht_r[:], ht[:])

            # ---- A2C head: hh = relu(W_ih @ h_t + b_ih) ---------------
            # moving-operand form: p0/p1 banks reused, 16 N=512 matmuls
            for c in range(8):
                nc.tensor.matmul(p0[:], ht_r[:, c:c + 1],
                                 wiht_sb[:, c, 0:512],
                                 start=(c == 0), stop=(c == 7))
                nc.tensor.matmul(p1[:], ht_r[:, c:c + 1],
                                 wiht_sb[:, c, 512:1024],
                                 start=(c == 0), stop=(c == 7))
            hh_row = cp.tile([1, H], F32)
            nc.vector.tensor_copy(hh_row[0:1, 0:512], p0[:])
            nc.vector.tensor_copy(hh_row[0:1, 512:1024], p1[:])
            for n in range(8):
                nc.tensor.transpose(psum_mt[:, n:n + 1],
                                    hh_row[0:1, n * 128:(n + 1) * 128],
                                    ones_sb[0:1, 0:1])
            hhb_sb = cp.tile([128, 8], F32)
            nc.vector.tensor_add(hhb_sb[:], psum_mt[:], biht_sb[:])
            hh_sb = cp.tile([128, 8], F32)
            nc.scalar.activation(hh_sb[:], hhb_sb[:], AF.Relu)

            psum_av = pp.tile([1, 3], F32, tag="pre0")
            for c in range(8):
                nc.tensor.matmul(psum_av[:], hh_sb[:, c:c + 1],
                                 wact_sb[:, c * 3:(c + 1) * 3],
                                 start=(c == 0), stop=(c == 7))
            av = cp.tile([1, 3], F32)
            nc.vector.tensor_add(av[:], psum_av[:], bac_sb[:])

            # ---- outputs ----------------------------------------------
            out_sb = cp.tile([128, 16], F32)
            nc.vector.tensor_copy(out_sb[:, 0:8], ht[:])
            nc.vector.tensor_copy(out_sb[:, 8:16], ct[:])
            nc.sync.dma_start(out_hc[:], out_sb[:])
            nc.sync.dma_start(out_av[:], av[:])

    nc.compile()
    return nc


def _get_nc():
    if "nc" not in _CACHE:
        _CACHE["nc"] = _build()
    return _CACHE["nc"]


def _prep_in_maps(x_t, h, c, keys, vals, W_i2h, b_i2h, W_h2h, b_h2h,
                  W_ih, b_ih, W_actor, b_actor, W_critic, b_critic, pick_arm):
    f = np.float32
    x_t = np.asarray(x_t, f)
    h = np.asarray(h, f).reshape(-1)          # [H]
    c = np.asarray(c, f).reshape(-1)          # [H]
    keys = np.asarray(keys, f)
    vals = np.asarray(vals, f)

    pa = int(np.asarray(pick_arm))
    start = min(max(pa * RD, 0), IN_DIM - RD)  # jax dynamic_slice clamping
    q = x_t[0, start:start + RD]

    q_rep = np.ascontiguousarray(
        np.broadcast_to(np.tile(q, G), (128, G * RD)))

    b5 = (np.asarray(b_i2h, f) + np.asarray(b_h2h, f))
    b5t = np.ascontiguousarray(b5.reshape(40, 128).T)
    biht = np.ascontiguousarray(np.asarray(b_ih, f).reshape(8, 128).T)
    c2t = np.ascontiguousarray(c.reshape(8, 128).T)

    BF = ml_dtypes.bfloat16
    wiht = np.ascontiguousarray(
        np.asarray(W_ih, f).T.reshape(8, 128, H).transpose(1, 0, 2)
        .reshape(128, 8 * H)).astype(BF)
    wac = np.vstack([np.asarray(W_actor, f), np.asarray(W_critic, f)])  # [3,H]
    wact = np.ascontiguousarray(
        wac.T.reshape(8, 128, 3).transpose(1, 0, 2).reshape(128, 24))
    bac = np.concatenate([np.asarray(b_actor, f),
                          np.asarray(b_critic, f)]).reshape(1, 3)

    W_i2hT = np.ascontiguousarray(np.asarray(W_i2h, f).T)
    wxt_zero = np.zeros_like(W_i2hT)
    x_col = np.ascontiguousarray(x_t[0].reshape(IN_DIM, 1))
    x_zero = np.zeros_like(x_col)

    in_maps = []
    for k in range(N_CORES):
        r0 = k * PER
        r1 = min(r0 + PER, D)
        n_valid = r1 - r0

        vals_p = np.zeros((PER, H), f)
        vals_p[:n_valid] = vals[r0:r1]
        vals_s = np.ascontiguousarray(
            vals_p.reshape(G, 128, H).transpose(1, 0, 2).reshape(128, G * H))
        keys_p = np.zeros((PER, RD), f)
        keys_p[:n_valid] = keys[r0:r1]
        keys_t = np.ascontiguousarray(
            keys_p.reshape(G, 128, RD).transpose(1, 0, 2).reshape(128, G * RD))
        idx = np.arange(G)[None, :] * 128 + np.arange(128)[:, None]
        mask = (idx < n_valid).astype(f)

        wht = np.ascontiguousarray(
            np.asarray(W_h2h, f)[:, k * 128:(k + 1) * 128].T)
        h_col = np.ascontiguousarray(h[k * 128:(k + 1) * 128].reshape(128, 1))

        in_maps.append({
            "vals_s": vals_s,
            "keys_t": keys_t,
            "q_rep": q_rep,
            "mask": mask,
            "wht": wht,
            "wxt": W_i2hT if k == 0 else wxt_zero,
            "x_col": x_col if k == 0 else x_zero,
            "h_col": h_col,
            "c2t": c2t,
            "b5t": b5t,
            "biht": biht,
            "wiht": wiht,
            "wact": wact,
            "bac": bac,
        })
    return in_maps


def _postprocess(out_hc, out_av):
    h_t = np.ascontiguousarray(out_hc[:, 0:8].T).reshape(-1)
    c_t = np.ascontiguousarray(out_hc[:, 8:16].T).reshape(-1)
    logits = out_av[0, 0:2].astype(np.float32)
    v = np.float32(out_av[0, 2])
    m = logits.max()
    ex = np.exp(logits - m)
    pi = (ex / ex.sum()).astype(np.float32)
    a = int(np.argmax(np.log(pi) + GUMBEL))
    logp = np.float32(np.log(pi[a]))
    return np.concatenate([pi, [v], [logp], h_t, c_t]).astype(np.float32)


def kernel(**inputs) -> np.ndarray:
    nc = _get_nc()
    in_maps = _prep_in_maps(**inputs)
    res = run_bass_kernel_spmd(
        nc, in_maps, core_ids=list(range(N_CORES)),
        **_CACHE.get("run_kwargs", {}))
    _CACHE["last_results"] = res
    r0 = res.results[0]
    return _postprocess(r0["out_hc"], r0["out_av"])


# revision 15
# speedup vs baseline: 1.1350x; 1.0090x over previous
"""Trainium2 Bass kernel for CompositionalTwoArmedAgent (DND-LSTM A2C step).

Strategy (8 NeuronCores, SPMD + AllReduce):
  - DND keys/vals tables sharded row-wise: 12544 rows/core (core 7 padded).
  - Cosine similarities are bounded in [-1, 1], so the softmax needs no
    max pass: each core computes e_i = exp(cos_i), a partial sum S_k and a
    partial weighted value sum p_k = e @ vals_k (TensorE, fp32r full rate).
  - The LSTM i2h/h2h GEMM is sharded over its contraction dim (128 h-dims
    per core; the x_t @ W_i2h.T part is zero-fed on cores 1..7).
  - Two AllReduces: [preact(5120) | S(1)] early (hidden under the vals
    stream, so the gate math is precomputed), p(1024) late (4 KB).
  - Every core then computes the identical tiny LSTM/A2C tail; host reads
    core 0's output, applies the 2-class softmax / fixed-key categorical
    sample, and packs the reference's output layout.
"""

import ml_dtypes
import numpy as np

import concourse.bacc as bacc
import concourse.bass as bass
import concourse.mybir as mybir
import concourse.tile as tile
from concourse.bass_utils import run_bass_kernel_spmd

N_CORES = 8
D, RD, H, IN_DIM, A = 100000, 10, 1024, 14, 2
PER = 12544            # padded rows per core = G * 128
G = 98                 # 128-row chunks per core
BLOCKS = [7] * 13 + [4, 2, 1]   # chunks per vals DMA block (descending tail)
F32 = mybir.dt.float32
F32R = mybir.dt.float32r
BF16 = mybir.dt.bfloat16

# jax.random.gumbel(jax.random.key(1), (2,), float32) — fixed constants of the
# reference's categorical sample (verified against jax.random.categorical).
GUMBEL = np.array([0.5325072, -0.01641824], np.float32)

_CACHE = {}


def _input_specs():
    return [
        ("vals_s", [128, G * H], F32R),    # row-chunk-tiled vals shard
        ("keys_t", [128, G * RD], F32),
        ("q_rep", [128, G * RD], F32),
        ("mask", [128, G], F32),
        ("wht", [128, 5 * H], F32R),
        ("wxt", [IN_DIM, 5 * H], F32R),
        ("x_col", [IN_DIM, 1], F32R),
        ("h_col", [128, 1], F32R),
        ("c2t", [128, 8], F32),
        ("b5t", [128, 40], F32),
        ("biht", [128, 8], F32),
        ("wiht", [128, 8 * H], BF16),
        ("wact", [128, 24], F32),
        ("bac", [1, 3], F32),
    ]


def _build():
    nc = bacc.Bacc("TRN2", target_bir_lowering=False, debug=False,
                   num_devices=N_CORES)
    d = {name: nc.dram_tensor(name, shp, dt, kind="ExternalInput")
         for name, shp, dt in _input_specs()}
    out_hc = nc.dram_tensor("out_hc", [128, 16], F32, kind="ExternalOutput")
    out_av = nc.dram_tensor("out_av", [1, 3], F32, kind="ExternalOutput")

    AF = mybir.ActivationFunctionType
    OP = mybir.AluOpType

    with tile.TileContext(nc) as tc:
        with (
            tc.tile_pool(name="const", bufs=1) as cp,
            tc.tile_pool(name="vals", bufs=3) as vp,
            tc.tile_pool(name="ps", bufs=1, space="PSUM") as pp,
            tc.tile_pool(name="dram", bufs=1, space="DRAM") as dp,
        ):
            # ---- persistent loads -------------------------------------
            keys_sb = cp.tile([128, G * RD], F32)
            q_sb = cp.tile([128, G * RD], F32)
            mask_sb = cp.tile([128, G], F32)
            wht_sb = cp.tile([128, 5 * H], F32R)
            wxt_sb = cp.tile([IN_DIM, 5 * H], F32R)
            x_col_sb = cp.tile([IN_DIM, 1], F32R)
            h_col_sb = cp.tile([128, 1], F32R)
            c2t_sb = cp.tile([128, 8], F32)
            b5t_sb = cp.tile([128, 40], F32)
            biht_sb = cp.tile([128, 8], F32)
            wiht_sb = cp.tile([128, 8, H], BF16)
            wact_sb = cp.tile([128, 24], F32)
            bac_sb = cp.tile([1, 3], F32)
            for name, t in [("wht", wht_sb), ("wxt", wxt_sb),
                            ("x_col", x_col_sb), ("h_col", h_col_sb),
                            ("keys_t", keys_sb), ("q_rep", q_sb),
                            ("mask", mask_sb), ("c2t", c2t_sb),
                            ("b5t", b5t_sb), ("biht", biht_sb),
                            ("wact", wact_sb), ("bac", bac_sb)]:
                nc.scalar.dma_start(t[:], d[name][:])
            nc.scalar.dma_start(
                wiht_sb[:], d["wiht"][:].rearrange("p (c j) -> p c j", j=H))

            ones_sb = cp.tile([128, 128], F32)
            nc.vector.memset(ones_sb[:], 1.0)

            # ---- ||q||^2 broadcast to all partitions ------------------
            sq_q = cp.tile([1, RD], F32)
            nc.scalar.activation(sq_q[:], q_sb[0:1, 0:RD], AF.Square)
            qnsq = cp.tile([1, 1], F32)
            nc.vector.reduce_sum(qnsq[:], sq_q[:], axis=mybir.AxisListType.X)
            psum_qn = pp.tile([128, 1], F32, tag="ps_small")
            nc.tensor.matmul(psum_qn[:], ones_sb[0:1, :], qnsq[:])
            qn2b = cp.tile([128, 1], F32)
            nc.vector.tensor_copy(qn2b[:], psum_qn[:])

            # ---- cosine sims -> masked exp weights --------------------
            prod = cp.tile([128, G * RD], F32)
            nc.vector.tensor_mul(prod[:], keys_sb[:], q_sb[:])
            dots = cp.tile([128, G], F32)
            nc.vector.tensor_reduce(
                dots[:], prod[:].rearrange("p (g r) -> p g r", r=RD),
                axis=mybir.AxisListType.X, op=OP.add)
            sqk = cp.tile([128, G * RD], F32)
            nc.scalar.activation(sqk[:], keys_sb[:], AF.Square)
            nsq = cp.tile([128, G], F32)
            nc.vector.tensor_reduce(
                nsq[:], sqk[:].rearrange("p (g r) -> p g r", r=RD),
                axis=mybir.AxisListType.X, op=OP.add)
            d2 = cp.tile([128, G], F32)
            nc.vector.tensor_scalar(d2[:], nsq[:], qn2b[:, 0:1], None, OP.mult)
            den = cp.tile([128, G], F32)
            nc.scalar.activation(den[:], d2[:], AF.Sqrt)
            denc = cp.tile([128, G], F32)
            nc.vector.tensor_scalar_max(denc[:], den[:], 1e-8)
            rec = cp.tile([128, G], F32)
            nc.vector.reciprocal(rec[:], denc[:])
            s_sb = cp.tile([128, G], F32)
            nc.vector.tensor_mul(s_sb[:], dots[:], rec[:])
            eraw = cp.tile([128, G], F32)
            nc.scalar.activation(eraw[:], s_sb[:], AF.Exp)
            e_sb = cp.tile([128, G], F32)
            rowsum = cp.tile([128, 1], F32)
            nc.vector.scalar_tensor_tensor(
                e_sb[:], eraw[:], 1.0, mask_sb[:], OP.mult, OP.mult,
                accum_out=rowsum[:])
            e_r = cp.tile([128, G], F32R)
            nc.vector.tensor_copy(e_r[:], e_sb[:])

            # ---- preact partial: [x;h_chunk] @ [WxT;WhT] --------------
            # moving-operand fp32r form: 20 N=512 matmuls into [1,512] rows,
            # then 40 PE transposes back to the compact [128, 40] col layout.
            psum_pre = pp.tile([128, 40], F32)
            for n in range(10):
                pre_ps = pp.tile([1, 512], F32, tag=f"pre{n % 2}")
                nc.tensor.matmul(pre_ps[:], h_col_sb[:],
                                 wht_sb[:, n * 512:(n + 1) * 512],
                                 start=True, stop=False)
                nc.tensor.matmul(pre_ps[:], x_col_sb[:],
                                 wxt_sb[:, n * 512:(n + 1) * 512],
                                 start=False, stop=True)
                row_scr = cp.tile([1, 512], F32, tag="rowscr", bufs=2)
                nc.vector.tensor_copy(row_scr[:], pre_ps[:])
                for t in range(4):
                    nc.tensor.transpose(psum_pre[:, 4 * n + t:4 * n + t + 1],
                                        row_scr[0:1, t * 128:(t + 1) * 128],
                                        ones_sb[0:1, 0:1])

            # ---- AllReduce #1: [preact(40) | S(1)] (hidden) -----------
            stage1 = cp.tile([128, 41], F32)
            nc.vector.tensor_copy(stage1[:, 0:40], psum_pre[:])
            nc.vector.tensor_copy(stage1[:, 40:41], rowsum[:])
            cc1_in = dp.tile([128, 41], F32)
            cc1_out = dp.tile([128, 41], F32, addr_space="Shared")
            nc.gpsimd.dma_start(cc1_in[:], stage1[:])
            nc.gpsimd.collective_compute(
                "AllReduce", OP.add,
                replica_groups=[list(range(N_CORES))],
                ins=[cc1_in[:]], outs=[cc1_out[:]])
            stage1o = cp.tile([128, 41], F32)
            nc.gpsimd.dma_start(stage1o[:], cc1_out[:])

            # ---- big matvec: p = e @ vals (fp32r, streamed) -----------
            p0 = pp.tile([1, 512], F32)
            p1 = pp.tile([1, 512], F32)
            g = 0
            for nb in BLOCKS:
                v = vp.tile([128, nb, H], F32R, tag="v")
                src = d["vals_s"][:, g * H:(g + nb) * H]
                nc.sync.dma_start(v[:], src.rearrange("p (c h) -> p c h", h=H))
                for c in range(nb):
                    e_col = e_r[:, g:g + 1]
                    nc.tensor.matmul(p0[:], e_col, v[:, c, 0:512],
                                     start=(g == 0), stop=(g == G - 1))
                    nc.tensor.matmul(p1[:], e_col, v[:, c, 512:1024],
                                     start=(g == 0), stop=(g == G - 1))
                    g += 1

            # ---- transpose p to [128, 8] ------------------------------
            p_sb = cp.tile([1, H], F32)
            nc.vector.tensor_copy(p_sb[0:1, 0:512], p0[:])
            nc.vector.tensor_copy(p_sb[0:1, 512:1024], p1[:])
            psum_mt = pp.tile([128, 8], F32)
            for n in range(8):
                nc.tensor.transpose(psum_mt[:, n:n + 1],
                                    p_sb[0:1, n * 128:(n + 1) * 128],
                                    ones_sb[0:1, 0:1])

            # ---- AllReduce #2: p (4 KB) -------------------------------
            stage2 = cp.tile([128, 8], F32)
            nc.vector.tensor_copy(stage2[:], psum_mt[:])
            cc2_in = dp.tile([128, 8], F32)
            cc2_out = dp.tile([128, 8], F32, addr_space="Shared")
            nc.sync.dma_start(cc2_in[:], stage2[:])
            nc.gpsimd.collective_compute(
                "AllReduce", OP.add,
                replica_groups=[list(range(N_CORES))],
                ins=[cc2_in[:]], outs=[cc2_out[:]])
            # ---- gate math from AR1 (hidden under the vals stream) ----
            prefull = cp.tile([128, 40], F32)
            nc.vector.tensor_add(prefull[:], stage1o[:, 0:40], b5t_sb[:])
            th = cp.tile([128, 32], F32)
            nc.scalar.activation(th[:], prefull[:, 0:32], AF.Tanh, scale=0.5)
            gates = cp.tile([128, 32], F32)
            nc.vector.tensor_scalar(gates[:], th[:], 0.5, 0.5, OP.mult, OP.add)
            cnew = cp.tile([128, 8], F32)
            nc.scalar.activation(cnew[:], prefull[:, 32:40], AF.Tanh)
            S_all = cp.tile([128, 1], F32)
            nc.gpsimd.partition_all_reduce(
                S_all[:], stage1o[:, 40:41], 128,
                bass.bass_isa.ReduceOp.add)
            invS = cp.tile([128, 1], F32)
            nc.vector.reciprocal(invS[:], S_all[:])
            t1 = cp.tile([128, 8], F32)
            nc.vector.tensor_mul(t1[:], gates[:, 0:8], c2t_sb[:])
            t2 = cp.tile([128, 8], F32)
            nc.vector.tensor_mul(t2[:], gates[:, 8:16], cnew[:])
            ct0 = cp.tile([128, 8], F32)
            nc.vector.tensor_add(ct0[:], t1[:], t2[:])

            stage2o = cp.tile([128, 8], F32)
            nc.sync.dma_start(stage2o[:], cc2_out[:])

            # ---- LSTM tail --------------------------------------------
            mt_sb = cp.tile([128, 8], F32)
            nc.scalar.activation(mt_sb[:], stage2o[:], AF.Tanh,
                                 scale=invS[:, 0:1])
            t3 = cp.tile([128, 8], F32)
            nc.vector.tensor_mul(t3[:], gates[:, 24:32], mt_sb[:])
            ct = cp.tile([128, 8], F32)
            nc.vector.tensor_add(ct[:], ct0[:], t3[:])
            tct = cp.tile([128, 8], F32)
            nc.scalar.activation(tct[:], ct[:], AF.Tanh)
            ht = cp.tile([128, 8], F32)
            nc.vector.tensor_mul(ht[:], gates[:, 16:24], tct[:])
            ht_r = cp.tile([128, 8], BF16)
            nc.vector.tensor_copy(ht_r[:], ht[:])

            # ---- A2C head: hh = relu(W_ih @ h_t + b_ih) ---------------
            # moving-operand form: p0/p1 banks reused, 16 N=512 matmuls
            for c in range(8):
                nc.tensor.matmul(p0[:], ht_r[:, c:c + 1],
                                 wiht_sb[:, c, 0:512],
                                 start=(c == 0), stop=(c == 7))
                nc.tensor.matmul(p1[:], ht_r[:, c:c + 1],
                                 wiht_sb[:, c, 512:1024],
                                 start=(c == 0), stop=(c == 7))
            hh_row = cp.tile([1, H], F32)
            nc.vector.tensor_copy(hh_row[0:1, 0:512], p0[:])
            nc.vector.tensor_copy(hh_row[0:1, 512:1024], p1[:])
            for n in range(8):
                nc.tensor.transpose(psum_mt[:, n:n + 1],
                                    hh_row[0:1, n * 128:(n + 1) * 128],
                                    ones_sb[0:1, 0:1])
            hhb_sb = cp.tile([128, 8], F32)
            nc.vector.tensor_add(hhb_sb[:], psum_mt[:], biht_sb[:])
            hh_sb = cp.tile([128, 8], F32)
            nc.scalar.activation(hh_sb[:], hhb_sb[:], AF.Relu)

            psum_av = pp.tile([1, 3], F32, tag="pre0")
            for c in range(8):
                nc.tensor.matmul(psum_av[:], hh_sb[:, c:c + 1],
                                 wact_sb[:, c * 3:(c + 1) * 3],
                                 start=(c == 0), stop=(c == 7))
            av = cp.tile([1, 3], F32)
            nc.vector.tensor_add(av[:], psum_av[:], bac_sb[:])

            # ---- outputs ----------------------------------------------
            out_sb = cp.tile([128, 16], F32)
            nc.vector.tensor_copy(out_sb[:, 0:8], ht[:])
            nc.vector.tensor_copy(out_sb[:, 8:16], ct[:])
            nc.sync.dma_start(out_hc[:], out_sb[:])
            nc.sync.dma_start(out_av[:], av[:])

    nc.compile()
    return nc


def _get_nc():
    if "nc" not in _CACHE:
        _CACHE["nc"] = _build()
    return _CACHE["nc"]


def _prep_in_maps(x_t, h, c, keys, vals, W_i2h, b_i2h, W_h2h, b_h2h,
                  W_ih, b_ih, W_actor, b_actor, W_critic, b_critic, pick_arm):
    f = np.float32
    x_t = np.asarray(x_t, f)
    h = np.asarray(h, f).reshape(-1)          # [H]
    c = np.asarray(c, f).reshape(-1)          # [H]
    keys = np.asarray(keys, f)
    vals = np.asarray(vals, f)

    pa = int(np.asarray(pick_arm))
    start = min(max(pa * RD, 0), IN_DIM - RD)  # jax dynamic_slice clamping
    q = x_t[0, start:start + RD]

    q_rep = np.ascontiguousarray(
        np.broadcast_to(np.tile(q, G), (128, G * RD)))

    b5 = (np.asarray(b_i2h, f) + np.asarray(b_h2h, f))
    b5t = np.ascontiguousarray(b5.reshape(40, 128).T)
    biht = np.ascontiguousarray(np.asarray(b_ih, f).reshape(8, 128).T)
    c2t = np.ascontiguousarray(c.reshape(8, 128).T)

    BF = ml_dtypes.bfloat16
    wiht = np.ascontiguousarray(
        np.asarray(W_ih, f).T.reshape(8, 128, H).transpose(1, 0, 2)
        .reshape(128, 8 * H)).astype(BF)
    wac = np.vstack([np.asarray(W_actor, f), np.asarray(W_critic, f)])  # [3,H]
    wact = np.ascontiguousarray(
        wac.T.reshape(8, 128, 3).transpose(1, 0, 2).reshape(128, 24))
    bac = np.concatenate([np.asarray(b_actor, f),
                          np.asarray(b_critic, f)]).reshape(1, 3)

    W_i2hT = np.ascontiguousarray(np.asarray(W_i2h, f).T)
    wxt_zero = np.zeros_like(W_i2hT)
    x_col = np.ascontiguousarray(x_t[0].reshape(IN_DIM, 1))
    x_zero = np.zeros_like(x_col)

    in_maps = []
    for k in range(N_CORES):
        r0 = k * PER
        r1 = min(r0 + PER, D)
        n_valid = r1 - r0

        vals_p = np.zeros((PER, H), f)
        vals_p[:n_valid] = vals[r0:r1]
        vals_s = np.ascontiguousarray(
            vals_p.reshape(G, 128, H).transpose(1, 0, 2).reshape(128, G * H))
        keys_p = np.zeros((PER, RD), f)
        keys_p[:n_valid] = keys[r0:r1]
        keys_t = np.ascontiguousarray(
            keys_p.reshape(G, 128, RD).transpose(1, 0, 2).reshape(128, G * RD))
        idx = np.arange(G)[None, :] * 128 + np.arange(128)[:, None]
        mask = (idx < n_valid).astype(f)

        wht = np.ascontiguousarray(
            np.asarray(W_h2h, f)[:, k * 128:(k + 1) * 128].T)
        h_col = np.ascontiguousarray(h[k * 128:(k + 1) * 128].reshape(128, 1))

        in_maps.append({
            "vals_s": vals_s,
            "keys_t": keys_t,
            "q_rep": q_rep,
            "mask": mask,
            "wht": wht,
            "wxt": W_i2hT if k == 0 else wxt_zero,
            "x_col": x_col if k == 0 else x_zero,
            "h_col": h_col,
            "c2t": c2t,
            "b5t": b5t,
            "biht": biht,
            "wiht": wiht,
            "wact": wact,
            "bac": bac,
        })
    return in_maps


def _postprocess(out_hc, out_av):
    h_t = np.ascontiguousarray(out_hc[:, 0:8].T).reshape(-1)
    c_t = np.ascontiguousarray(out_hc[:, 8:16].T).reshape(-1)
    logits = out_av[0, 0:2].astype(np.float32)
    v = np.float32(out_av[0, 2])
    m = logits.max()
    ex = np.exp(logits - m)
    pi = (ex / ex.sum()).astype(np.float32)
    a = int(np.argmax(np.log(pi) + GUMBEL))
    logp = np.float32(np.log(pi[a]))
    return np.concatenate([pi, [v], [logp], h_t, c_t]).astype(np.float32)


def kernel(**inputs) -> np.ndarray:
    nc = _get_nc()
    in_maps = _prep_in_maps(**inputs)
    res = run_bass_kernel_spmd(
        nc, in_maps, core_ids=list(range(N_CORES)),
        **_CACHE.get("run_kwargs", {}))
    _CACHE["last_results"] = res
    r0 = res.results[0]
    return _postprocess(r0["out_hc"], r0["out_av"])


# revision 16
# speedup vs baseline: 1.6133x; 1.4214x over previous
"""Trainium2 Bass kernel for CompositionalTwoArmedAgent (DND-LSTM A2C step).

Strategy (8 NeuronCores, SPMD + AllReduce):
  - DND keys/vals tables sharded row-wise: 12544 rows/core (core 7 padded).
  - Cosine similarities are bounded in [-1, 1], so the softmax needs no
    max pass: each core computes e_i = exp(cos_i), a partial sum S_k and a
    partial weighted value sum p_k = e @ vals_k (TensorE, fp32r full rate).
  - The LSTM i2h/h2h GEMM is sharded over its contraction dim (128 h-dims
    per core; the x_t @ W_i2h.T part is zero-fed on cores 1..7).
  - Two AllReduces: [preact(5120) | S(1)] early (hidden under the vals
    stream, so the gate math is precomputed), p(1024) late (4 KB).
  - Every core then computes the identical tiny LSTM/A2C tail; host reads
    core 0's output, applies the 2-class softmax / fixed-key categorical
    sample, and packs the reference's output layout.
"""

import ml_dtypes
import numpy as np

import concourse.bacc as bacc
import concourse.bass as bass
import concourse.mybir as mybir
import concourse.tile as tile
from concourse.bass_utils import run_bass_kernel_spmd

N_CORES = 8
D, RD, H, IN_DIM, A = 100000, 10, 1024, 14, 2
PER = 12544            # padded rows per core = G * 128
G = 98                 # 128-row chunks per core
BLOCKS = [7] * 13 + [4, 2, 1]   # chunks per vals DMA block (descending tail)
F32 = mybir.dt.float32
F32R = mybir.dt.float32r
BF16 = mybir.dt.bfloat16

# jax.random.gumbel(jax.random.key(1), (2,), float32) — fixed constants of the
# reference's categorical sample (verified against jax.random.categorical).
GUMBEL = np.array([0.5325072, -0.01641824], np.float32)

_CACHE = {}


def _input_specs():
    return [
        ("vals_s", [128, G * H], BF16),    # row-chunk-tiled vals shard
        ("keys_t", [128, G * RD], F32),
        ("q_rep", [128, G * RD], F32),
        ("mask", [128, G], F32),
        ("wht", [128, 5 * H], F32R),
        ("wxt", [IN_DIM, 5 * H], F32R),
        ("x_col", [IN_DIM, 1], F32R),
        ("h_col", [128, 1], F32R),
        ("c2t", [128, 8], F32),
        ("b5t", [128, 40], F32),
        ("biht", [128, 8], F32),
        ("wiht", [128, 8 * H], BF16),
        ("wact", [128, 24], F32),
        ("bac", [1, 3], F32),
    ]


def _build():
    nc = bacc.Bacc("TRN2", target_bir_lowering=False, debug=False,
                   num_devices=N_CORES)
    d = {name: nc.dram_tensor(name, shp, dt, kind="ExternalInput")
         for name, shp, dt in _input_specs()}
    out_hc = nc.dram_tensor("out_hc", [128, 16], F32, kind="ExternalOutput")
    out_av = nc.dram_tensor("out_av", [1, 3], F32, kind="ExternalOutput")

    AF = mybir.ActivationFunctionType
    OP = mybir.AluOpType

    with tile.TileContext(nc) as tc:
        with (
            tc.tile_pool(name="const", bufs=1) as cp,
            tc.tile_pool(name="vals", bufs=4) as vp,
            tc.tile_pool(name="ps", bufs=1, space="PSUM") as pp,
            tc.tile_pool(name="dram", bufs=1, space="DRAM") as dp,
        ):
            # ---- persistent loads -------------------------------------
            keys_sb = cp.tile([128, G * RD], F32)
            q_sb = cp.tile([128, G * RD], F32)
            mask_sb = cp.tile([128, G], F32)
            wht_sb = cp.tile([128, 5 * H], F32R)
            wxt_sb = cp.tile([IN_DIM, 5 * H], F32R)
            x_col_sb = cp.tile([IN_DIM, 1], F32R)
            h_col_sb = cp.tile([128, 1], F32R)
            c2t_sb = cp.tile([128, 8], F32)
            b5t_sb = cp.tile([128, 40], F32)
            biht_sb = cp.tile([128, 8], F32)
            wiht_sb = cp.tile([128, 8, H], BF16)
            wact_sb = cp.tile([128, 24], F32)
            bac_sb = cp.tile([1, 3], F32)
            for name, t in [("wht", wht_sb), ("wxt", wxt_sb),
                            ("x_col", x_col_sb), ("h_col", h_col_sb),
                            ("keys_t", keys_sb), ("q_rep", q_sb),
                            ("mask", mask_sb), ("c2t", c2t_sb),
                            ("b5t", b5t_sb), ("biht", biht_sb),
                            ("wact", wact_sb), ("bac", bac_sb)]:
                nc.scalar.dma_start(t[:], d[name][:])
            nc.scalar.dma_start(
                wiht_sb[:], d["wiht"][:].rearrange("p (c j) -> p c j", j=H))

            ones_sb = cp.tile([128, 128], F32)
            nc.vector.memset(ones_sb[:], 1.0)

            # ---- ||q||^2 broadcast to all partitions ------------------
            sq_q = cp.tile([1, RD], F32)
            nc.scalar.activation(sq_q[:], q_sb[0:1, 0:RD], AF.Square)
            qnsq = cp.tile([1, 1], F32)
            nc.vector.reduce_sum(qnsq[:], sq_q[:], axis=mybir.AxisListType.X)
            psum_qn = pp.tile([128, 1], F32, tag="ps_small")
            nc.tensor.matmul(psum_qn[:], ones_sb[0:1, :], qnsq[:])
            qn2b = cp.tile([128, 1], F32)
            nc.vector.tensor_copy(qn2b[:], psum_qn[:])

            # ---- cosine sims -> masked exp weights --------------------
            prod = cp.tile([128, G * RD], F32)
            nc.vector.tensor_mul(prod[:], keys_sb[:], q_sb[:])
            dots = cp.tile([128, G], F32)
            nc.vector.tensor_reduce(
                dots[:], prod[:].rearrange("p (g r) -> p g r", r=RD),
                axis=mybir.AxisListType.X, op=OP.add)
            sqk = cp.tile([128, G * RD], F32)
            nc.scalar.activation(sqk[:], keys_sb[:], AF.Square)
            nsq = cp.tile([128, G], F32)
            nc.vector.tensor_reduce(
                nsq[:], sqk[:].rearrange("p (g r) -> p g r", r=RD),
                axis=mybir.AxisListType.X, op=OP.add)
            d2 = cp.tile([128, G], F32)
            nc.vector.tensor_scalar(d2[:], nsq[:], qn2b[:, 0:1], None, OP.mult)
            den = cp.tile([128, G], F32)
            nc.scalar.activation(den[:], d2[:], AF.Sqrt)
            denc = cp.tile([128, G], F32)
            nc.vector.tensor_scalar_max(denc[:], den[:], 1e-8)
            rec = cp.tile([128, G], F32)
            nc.vector.reciprocal(rec[:], denc[:])
            s_sb = cp.tile([128, G], F32)
            nc.vector.tensor_mul(s_sb[:], dots[:], rec[:])
            eraw = cp.tile([128, G], F32)
            nc.scalar.activation(eraw[:], s_sb[:], AF.Exp)
            e_sb = cp.tile([128, G], F32)
            rowsum = cp.tile([128, 1], F32)
            nc.vector.scalar_tensor_tensor(
                e_sb[:], eraw[:], 1.0, mask_sb[:], OP.mult, OP.mult,
                accum_out=rowsum[:])
            e_r = cp.tile([128, G], BF16)
            nc.vector.tensor_copy(e_r[:], e_sb[:])

            # ---- preact partial: [x;h_chunk] @ [WxT;WhT] --------------
            # moving-operand fp32r form: 20 N=512 matmuls into [1,512] rows,
            # then 40 PE transposes back to the compact [128, 40] col layout.
            psum_pre = pp.tile([128, 40], F32)
            for n in range(10):
                pre_ps = pp.tile([1, 512], F32, tag=f"pre{n % 2}")
                nc.tensor.matmul(pre_ps[:], h_col_sb[:],
                                 wht_sb[:, n * 512:(n + 1) * 512],
                                 start=True, stop=False)
                nc.tensor.matmul(pre_ps[:], x_col_sb[:],
                                 wxt_sb[:, n * 512:(n + 1) * 512],
                                 start=False, stop=True)
                row_scr = cp.tile([1, 512], F32, tag="rowscr", bufs=2)
                nc.vector.tensor_copy(row_scr[:], pre_ps[:])
                for t in range(4):
                    nc.tensor.transpose(psum_pre[:, 4 * n + t:4 * n + t + 1],
                                        row_scr[0:1, t * 128:(t + 1) * 128],
                                        ones_sb[0:1, 0:1])

            # ---- AllReduce #1: [preact(40) | S(1)] (hidden) -----------
            stage1 = cp.tile([128, 41], F32)
            nc.vector.tensor_copy(stage1[:, 0:40], psum_pre[:])
            nc.vector.tensor_copy(stage1[:, 40:41], rowsum[:])
            cc1_in = dp.tile([128, 41], F32)
            cc1_out = dp.tile([128, 41], F32, addr_space="Shared")
            nc.gpsimd.dma_start(cc1_in[:], stage1[:])
            nc.gpsimd.collective_compute(
                "AllReduce", OP.add,
                replica_groups=[list(range(N_CORES))],
                ins=[cc1_in[:]], outs=[cc1_out[:]])
            stage1o = cp.tile([128, 41], F32)
            nc.gpsimd.dma_start(stage1o[:], cc1_out[:])

            # ---- big matvec: p = e @ vals (fp32r, streamed) -----------
            p0 = pp.tile([1, 512], F32)
            p1 = pp.tile([1, 512], F32)
            g = 0
            for nb in BLOCKS:
                v = vp.tile([128, nb, H], BF16, tag="v")
                src = d["vals_s"][:, g * H:(g + nb) * H]
                nc.sync.dma_start(v[:], src.rearrange("p (c h) -> p c h", h=H))
                for c in range(nb):
                    e_col = e_r[:, g:g + 1]
                    nc.tensor.matmul(p0[:], e_col, v[:, c, 0:512],
                                     start=(g == 0), stop=(g == G - 1))
                    nc.tensor.matmul(p1[:], e_col, v[:, c, 512:1024],
                                     start=(g == 0), stop=(g == G - 1))
                    g += 1

            # ---- transpose p to [128, 8] ------------------------------
            p_sb = cp.tile([1, H], F32)
            nc.vector.tensor_copy(p_sb[0:1, 0:512], p0[:])
            nc.vector.tensor_copy(p_sb[0:1, 512:1024], p1[:])
            psum_mt = pp.tile([128, 8], F32)
            for n in range(8):
                nc.tensor.transpose(psum_mt[:, n:n + 1],
                                    p_sb[0:1, n * 128:(n + 1) * 128],
                                    ones_sb[0:1, 0:1])

            # ---- AllReduce #2: p (4 KB) -------------------------------
            stage2 = cp.tile([128, 8], F32)
            nc.vector.tensor_copy(stage2[:], psum_mt[:])
            cc2_in = dp.tile([128, 8], F32)
            cc2_out = dp.tile([128, 8], F32, addr_space="Shared")
            nc.sync.dma_start(cc2_in[:], stage2[:])
            nc.gpsimd.collective_compute(
                "AllReduce", OP.add,
                replica_groups=[list(range(N_CORES))],
                ins=[cc2_in[:]], outs=[cc2_out[:]])
            # ---- gate math from AR1 (hidden under the vals stream) ----
            prefull = cp.tile([128, 40], F32)
            nc.vector.tensor_add(prefull[:], stage1o[:, 0:40], b5t_sb[:])
            th = cp.tile([128, 32], F32)
            nc.scalar.activation(th[:], prefull[:, 0:32], AF.Tanh, scale=0.5)
            gates = cp.tile([128, 32], F32)
            nc.vector.tensor_scalar(gates[:], th[:], 0.5, 0.5, OP.mult, OP.add)
            cnew = cp.tile([128, 8], F32)
            nc.scalar.activation(cnew[:], prefull[:, 32:40], AF.Tanh)
            S_all = cp.tile([128, 1], F32)
            nc.gpsimd.partition_all_reduce(
                S_all[:], stage1o[:, 40:41], 128,
                bass.bass_isa.ReduceOp.add)
            invS = cp.tile([128, 1], F32)
            nc.vector.reciprocal(invS[:], S_all[:])
            t1 = cp.tile([128, 8], F32)
            nc.vector.tensor_mul(t1[:], gates[:, 0:8], c2t_sb[:])
            t2 = cp.tile([128, 8], F32)
            nc.vector.tensor_mul(t2[:], gates[:, 8:16], cnew[:])
            ct0 = cp.tile([128, 8], F32)
            nc.vector.tensor_add(ct0[:], t1[:], t2[:])

            stage2o = cp.tile([128, 8], F32)
            nc.sync.dma_start(stage2o[:], cc2_out[:])

            # ---- LSTM tail --------------------------------------------
            mt_sb = cp.tile([128, 8], F32)
            nc.scalar.activation(mt_sb[:], stage2o[:], AF.Tanh,
                                 scale=invS[:, 0:1])
            t3 = cp.tile([128, 8], F32)
            nc.vector.tensor_mul(t3[:], gates[:, 24:32], mt_sb[:])
            ct = cp.tile([128, 8], F32)
            nc.vector.tensor_add(ct[:], ct0[:], t3[:])
            tct = cp.tile([128, 8], F32)
            nc.scalar.activation(tct[:], ct[:], AF.Tanh)
            ht = cp.tile([128, 8], F32)
            nc.vector.tensor_mul(ht[:], gates[:, 16:24], tct[:])
            ht_r = cp.tile([128, 8], BF16)
            nc.vector.tensor_copy(ht_r[:], ht[:])

            # ---- A2C head: hh = relu(W_ih @ h_t + b_ih) ---------------
            # moving-operand form: p0/p1 banks reused, 16 N=512 matmuls
            for c in range(8):
                nc.tensor.matmul(p0[:], ht_r[:, c:c + 1],
                                 wiht_sb[:, c, 0:512],
                                 start=(c == 0), stop=(c == 7))
                nc.tensor.matmul(p1[:], ht_r[:, c:c + 1],
                                 wiht_sb[:, c, 512:1024],
                                 start=(c == 0), stop=(c == 7))
            hh_row = cp.tile([1, H], F32)
            nc.vector.tensor_copy(hh_row[0:1, 0:512], p0[:])
            nc.vector.tensor_copy(hh_row[0:1, 512:1024], p1[:])
            for n in range(8):
                nc.tensor.transpose(psum_mt[:, n:n + 1],
                                    hh_row[0:1, n * 128:(n + 1) * 128],
                                    ones_sb[0:1, 0:1])
            hhb_sb = cp.tile([128, 8], F32)
            nc.vector.tensor_add(hhb_sb[:], psum_mt[:], biht_sb[:])
            hh_sb = cp.tile([128, 8], F32)
            nc.scalar.activation(hh_sb[:], hhb_sb[:], AF.Relu)

            psum_av = pp.tile([1, 3], F32, tag="pre0")
            for c in range(8):
                nc.tensor.matmul(psum_av[:], hh_sb[:, c:c + 1],
                                 wact_sb[:, c * 3:(c + 1) * 3],
                                 start=(c == 0), stop=(c == 7))
            av = cp.tile([1, 3], F32)
            nc.vector.tensor_add(av[:], psum_av[:], bac_sb[:])

            # ---- outputs ----------------------------------------------
            out_sb = cp.tile([128, 16], F32)
            nc.vector.tensor_copy(out_sb[:, 0:8], ht[:])
            nc.vector.tensor_copy(out_sb[:, 8:16], ct[:])
            nc.sync.dma_start(out_hc[:], out_sb[:])
            nc.sync.dma_start(out_av[:], av[:])

    nc.compile()
    return nc


def _get_nc():
    if "nc" not in _CACHE:
        _CACHE["nc"] = _build()
    return _CACHE["nc"]


def _prep_in_maps(x_t, h, c, keys, vals, W_i2h, b_i2h, W_h2h, b_h2h,
                  W_ih, b_ih, W_actor, b_actor, W_critic, b_critic, pick_arm):
    f = np.float32
    x_t = np.asarray(x_t, f)
    h = np.asarray(h, f).reshape(-1)          # [H]
    c = np.asarray(c, f).reshape(-1)          # [H]
    keys = np.asarray(keys, f)
    vals = np.asarray(vals, f)

    pa = int(np.asarray(pick_arm))
    start = min(max(pa * RD, 0), IN_DIM - RD)  # jax dynamic_slice clamping
    q = x_t[0, start:start + RD]

    q_rep = np.ascontiguousarray(
        np.broadcast_to(np.tile(q, G), (128, G * RD)))

    b5 = (np.asarray(b_i2h, f) + np.asarray(b_h2h, f))
    b5t = np.ascontiguousarray(b5.reshape(40, 128).T)
    biht = np.ascontiguousarray(np.asarray(b_ih, f).reshape(8, 128).T)
    c2t = np.ascontiguousarray(c.reshape(8, 128).T)

    BF = ml_dtypes.bfloat16
    wiht = np.ascontiguousarray(
        np.asarray(W_ih, f).T.reshape(8, 128, H).transpose(1, 0, 2)
        .reshape(128, 8 * H)).astype(BF)
    wac = np.vstack([np.asarray(W_actor, f), np.asarray(W_critic, f)])  # [3,H]
    wact = np.ascontiguousarray(
        wac.T.reshape(8, 128, 3).transpose(1, 0, 2).reshape(128, 24))
    bac = np.concatenate([np.asarray(b_actor, f),
                          np.asarray(b_critic, f)]).reshape(1, 3)

    W_i2hT = np.ascontiguousarray(np.asarray(W_i2h, f).T)
    wxt_zero = np.zeros_like(W_i2hT)
    x_col = np.ascontiguousarray(x_t[0].reshape(IN_DIM, 1))
    x_zero = np.zeros_like(x_col)

    in_maps = []
    for k in range(N_CORES):
        r0 = k * PER
        r1 = min(r0 + PER, D)
        n_valid = r1 - r0

        vals_p = np.zeros((PER, H), f)
        vals_p[:n_valid] = vals[r0:r1]
        vals_s = np.ascontiguousarray(
            vals_p.reshape(G, 128, H).transpose(1, 0, 2)
            .reshape(128, G * H)).astype(BF)
        keys_p = np.zeros((PER, RD), f)
        keys_p[:n_valid] = keys[r0:r1]
        keys_t = np.ascontiguousarray(
            keys_p.reshape(G, 128, RD).transpose(1, 0, 2).reshape(128, G * RD))
        idx = np.arange(G)[None, :] * 128 + np.arange(128)[:, None]
        mask = (idx < n_valid).astype(f)

        wht = np.ascontiguousarray(
            np.asarray(W_h2h, f)[:, k * 128:(k + 1) * 128].T)
        h_col = np.ascontiguousarray(h[k * 128:(k + 1) * 128].reshape(128, 1))

        in_maps.append({
            "vals_s": vals_s,
            "keys_t": keys_t,
            "q_rep": q_rep,
            "mask": mask,
            "wht": wht,
            "wxt": W_i2hT if k == 0 else wxt_zero,
            "x_col": x_col if k == 0 else x_zero,
            "h_col": h_col,
            "c2t": c2t,
            "b5t": b5t,
            "biht": biht,
            "wiht": wiht,
            "wact": wact,
            "bac": bac,
        })
    return in_maps


def _postprocess(out_hc, out_av):
    h_t = np.ascontiguousarray(out_hc[:, 0:8].T).reshape(-1)
    c_t = np.ascontiguousarray(out_hc[:, 8:16].T).reshape(-1)
    logits = out_av[0, 0:2].astype(np.float32)
    v = np.float32(out_av[0, 2])
    m = logits.max()
    ex = np.exp(logits - m)
    pi = (ex / ex.sum()).astype(np.float32)
    a = int(np.argmax(np.log(pi) + GUMBEL))
    logp = np.float32(np.log(pi[a]))
    return np.concatenate([pi, [v], [logp], h_t, c_t]).astype(np.float32)


def kernel(**inputs) -> np.ndarray:
    nc = _get_nc()
    in_maps = _prep_in_maps(**inputs)
    res = run_bass_kernel_spmd(
        nc, in_maps, core_ids=list(range(N_CORES)),
        **_CACHE.get("run_kwargs", {}))
    _CACHE["last_results"] = res
    r0 = res.results[0]
    return _postprocess(r0["out_hc"], r0["out_av"])


# revision 17
# speedup vs baseline: 1.6683x; 1.0341x over previous
"""Trainium2 Bass kernel for CompositionalTwoArmedAgent (DND-LSTM A2C step).

Strategy (8 NeuronCores, SPMD + AllReduce):
  - DND keys/vals tables sharded row-wise: 12544 rows/core (core 7 padded).
  - Cosine similarities are bounded in [-1, 1], so the softmax needs no
    max pass: each core computes e_i = exp(cos_i), a partial sum S_k and a
    partial weighted value sum p_k = e @ vals_k (TensorE, fp32r full rate).
  - The LSTM i2h/h2h GEMM is sharded over its contraction dim (128 h-dims
    per core; the x_t @ W_i2h.T part is zero-fed on cores 1..7).
  - Two AllReduces: [preact(5120) | S(1)] early (hidden under the vals
    stream, so the gate math is precomputed), p(1024) late (4 KB).
  - Every core then computes the identical tiny LSTM/A2C tail; host reads
    core 0's output, applies the 2-class softmax / fixed-key categorical
    sample, and packs the reference's output layout.
"""

import ml_dtypes
import numpy as np

import concourse.bacc as bacc
import concourse.bass as bass
import concourse.mybir as mybir
import concourse.tile as tile
from concourse.bass_utils import run_bass_kernel_spmd

N_CORES = 8
D, RD, H, IN_DIM, A = 100000, 10, 1024, 14, 2
PER = 12544            # padded rows per core = G * 128
G = 98                 # 128-row chunks per core
BLOCKS = [14] * 6 + [7, 4, 2, 1]   # chunks per vals DMA block (descending tail)
F32 = mybir.dt.float32
F32R = mybir.dt.float32r
BF16 = mybir.dt.bfloat16

# jax.random.gumbel(jax.random.key(1), (2,), float32) — fixed constants of the
# reference's categorical sample (verified against jax.random.categorical).
GUMBEL = np.array([0.5325072, -0.01641824], np.float32)

_CACHE = {}


def _input_specs():
    return [
        ("vals_s", [128, G * H], BF16),    # row-chunk-tiled vals shard
        ("keys_t", [128, G * RD], F32),
        ("q_rep", [128, G * RD], F32),
        ("mask", [128, G], F32),
        ("wht", [128, 5 * H], F32R),
        ("wxt", [IN_DIM, 5 * H], F32R),
        ("x_col", [IN_DIM, 1], F32R),
        ("h_col", [128, 1], F32R),
        ("c2t", [128, 8], F32),
        ("b5t", [128, 40], F32),
        ("biht", [128, 8], F32),
        ("wiht", [128, 8 * H], BF16),
        ("wact", [128, 24], F32),
        ("bac", [1, 3], F32),
    ]


def _build():
    nc = bacc.Bacc("TRN2", target_bir_lowering=False, debug=False,
                   num_devices=N_CORES)
    d = {name: nc.dram_tensor(name, shp, dt, kind="ExternalInput")
         for name, shp, dt in _input_specs()}
    out_hc = nc.dram_tensor("out_hc", [128, 16], F32, kind="ExternalOutput")
    out_av = nc.dram_tensor("out_av", [1, 3], F32, kind="ExternalOutput")

    AF = mybir.ActivationFunctionType
    OP = mybir.AluOpType

    with tile.TileContext(nc) as tc:
        with (
            tc.tile_pool(name="const", bufs=1) as cp,
            tc.tile_pool(name="vals", bufs=3) as vp,
            tc.tile_pool(name="ps", bufs=1, space="PSUM") as pp,
            tc.tile_pool(name="dram", bufs=1, space="DRAM") as dp,
        ):
            # ---- persistent loads -------------------------------------
            keys_sb = cp.tile([128, G * RD], F32)
            q_sb = cp.tile([128, G * RD], F32)
            mask_sb = cp.tile([128, G], F32)
            wht_sb = cp.tile([128, 5 * H], F32R)
            wxt_sb = cp.tile([IN_DIM, 5 * H], F32R)
            x_col_sb = cp.tile([IN_DIM, 1], F32R)
            h_col_sb = cp.tile([128, 1], F32R)
            c2t_sb = cp.tile([128, 8], F32)
            b5t_sb = cp.tile([128, 40], F32)
            biht_sb = cp.tile([128, 8], F32)
            wiht_sb = cp.tile([128, 8, H], BF16)
            wact_sb = cp.tile([128, 24], F32)
            bac_sb = cp.tile([1, 3], F32)
            for name, t in [("wht", wht_sb), ("wxt", wxt_sb),
                            ("x_col", x_col_sb), ("h_col", h_col_sb),
                            ("keys_t", keys_sb), ("q_rep", q_sb),
                            ("mask", mask_sb), ("c2t", c2t_sb),
                            ("b5t", b5t_sb), ("biht", biht_sb),
                            ("wact", wact_sb), ("bac", bac_sb)]:
                nc.scalar.dma_start(t[:], d[name][:])
            nc.scalar.dma_start(
                wiht_sb[:], d["wiht"][:].rearrange("p (c j) -> p c j", j=H))

            ones_sb = cp.tile([128, 128], F32)
            nc.vector.memset(ones_sb[:], 1.0)

            # ---- ||q||^2 broadcast to all partitions ------------------
            sq_q = cp.tile([1, RD], F32)
            nc.scalar.activation(sq_q[:], q_sb[0:1, 0:RD], AF.Square)
            qnsq = cp.tile([1, 1], F32)
            nc.vector.reduce_sum(qnsq[:], sq_q[:], axis=mybir.AxisListType.X)
            psum_qn = pp.tile([128, 1], F32, tag="ps_small")
            nc.tensor.matmul(psum_qn[:], ones_sb[0:1, :], qnsq[:])
            qn2b = cp.tile([128, 1], F32)
            nc.vector.tensor_copy(qn2b[:], psum_qn[:])

            # ---- cosine sims -> masked exp weights --------------------
            prod = cp.tile([128, G * RD], F32)
            nc.vector.tensor_mul(prod[:], keys_sb[:], q_sb[:])
            dots = cp.tile([128, G], F32)
            nc.vector.tensor_reduce(
                dots[:], prod[:].rearrange("p (g r) -> p g r", r=RD),
                axis=mybir.AxisListType.X, op=OP.add)
            sqk = cp.tile([128, G * RD], F32)
            nc.scalar.activation(sqk[:], keys_sb[:], AF.Square)
            nsq = cp.tile([128, G], F32)
            nc.vector.tensor_reduce(
                nsq[:], sqk[:].rearrange("p (g r) -> p g r", r=RD),
                axis=mybir.AxisListType.X, op=OP.add)
            d2 = cp.tile([128, G], F32)
            nc.vector.tensor_scalar(d2[:], nsq[:], qn2b[:, 0:1], None, OP.mult)
            den = cp.tile([128, G], F32)
            nc.scalar.activation(den[:], d2[:], AF.Sqrt)
            denc = cp.tile([128, G], F32)
            nc.vector.tensor_scalar_max(denc[:], den[:], 1e-8)
            rec = cp.tile([128, G], F32)
            nc.vector.reciprocal(rec[:], denc[:])
            s_sb = cp.tile([128, G], F32)
            nc.vector.tensor_mul(s_sb[:], dots[:], rec[:])
            eraw = cp.tile([128, G], F32)
            nc.scalar.activation(eraw[:], s_sb[:], AF.Exp)
            e_sb = cp.tile([128, G], F32)
            rowsum = cp.tile([128, 1], F32)
            nc.vector.scalar_tensor_tensor(
                e_sb[:], eraw[:], 1.0, mask_sb[:], OP.mult, OP.mult,
                accum_out=rowsum[:])
            e_r = cp.tile([128, G], BF16)
            nc.vector.tensor_copy(e_r[:], e_sb[:])

            # ---- preact partial: [x;h_chunk] @ [WxT;WhT] --------------
            # moving-operand fp32r form: 20 N=512 matmuls into [1,512] rows,
            # then 40 PE transposes back to the compact [128, 40] col layout.
            psum_pre = pp.tile([128, 40], F32)
            for n in range(10):
                pre_ps = pp.tile([1, 512], F32, tag=f"pre{n % 2}")
                nc.tensor.matmul(pre_ps[:], h_col_sb[:],
                                 wht_sb[:, n * 512:(n + 1) * 512],
                                 start=True, stop=False)
                nc.tensor.matmul(pre_ps[:], x_col_sb[:],
                                 wxt_sb[:, n * 512:(n + 1) * 512],
                                 start=False, stop=True)
                row_scr = cp.tile([1, 512], F32, tag="rowscr", bufs=2)
                nc.vector.tensor_copy(row_scr[:], pre_ps[:])
                for t in range(4):
                    nc.tensor.transpose(psum_pre[:, 4 * n + t:4 * n + t + 1],
                                        row_scr[0:1, t * 128:(t + 1) * 128],
                                        ones_sb[0:1, 0:1])

            # ---- AllReduce #1: [preact(40) | S(1)] (hidden) -----------
            stage1 = cp.tile([128, 41], F32)
            nc.vector.tensor_copy(stage1[:, 0:40], psum_pre[:])
            nc.vector.tensor_copy(stage1[:, 40:41], rowsum[:])
            cc1_in = dp.tile([128, 41], F32)
            cc1_out = dp.tile([128, 41], F32, addr_space="Shared")
            nc.gpsimd.dma_start(cc1_in[:], stage1[:])
            nc.gpsimd.collective_compute(
                "AllReduce", OP.add,
                replica_groups=[list(range(N_CORES))],
                ins=[cc1_in[:]], outs=[cc1_out[:]])
            stage1o = cp.tile([128, 41], F32)
            nc.gpsimd.dma_start(stage1o[:], cc1_out[:])

            # ---- big matvec: p = e @ vals (fp32r, streamed) -----------
            p0 = pp.tile([1, 512], F32)
            p1 = pp.tile([1, 512], F32)
            g = 0
            for nb in BLOCKS:
                v = vp.tile([128, nb, H], BF16, tag="v")
                src = d["vals_s"][:, g * H:(g + nb) * H]
                nc.sync.dma_start(v[:], src.rearrange("p (c h) -> p c h", h=H))
                for c in range(nb):
                    e_col = e_r[:, g:g + 1]
                    nc.tensor.matmul(p0[:], e_col, v[:, c, 0:512],
                                     start=(g == 0), stop=(g == G - 1))
                    nc.tensor.matmul(p1[:], e_col, v[:, c, 512:1024],
                                     start=(g == 0), stop=(g == G - 1))
                    g += 1

            # ---- transpose p to [128, 8] ------------------------------
            p_sb = cp.tile([1, H], F32)
            nc.vector.tensor_copy(p_sb[0:1, 0:512], p0[:])
            nc.vector.tensor_copy(p_sb[0:1, 512:1024], p1[:])
            psum_mt = pp.tile([128, 8], F32)
            for n in range(8):
                nc.tensor.transpose(psum_mt[:, n:n + 1],
                                    p_sb[0:1, n * 128:(n + 1) * 128],
                                    ones_sb[0:1, 0:1])

            # ---- AllReduce #2: p (4 KB) -------------------------------
            stage2 = cp.tile([128, 8], F32)
            nc.vector.tensor_copy(stage2[:], psum_mt[:])
            cc2_in = dp.tile([128, 8], F32)
            cc2_out = dp.tile([128, 8], F32, addr_space="Shared")
            nc.sync.dma_start(cc2_in[:], stage2[:])
            nc.gpsimd.collective_compute(
                "AllReduce", OP.add,
                replica_groups=[list(range(N_CORES))],
                ins=[cc2_in[:]], outs=[cc2_out[:]])
            # ---- gate math from AR1 (hidden under the vals stream) ----
            prefull = cp.tile([128, 40], F32)
            nc.vector.tensor_add(prefull[:], stage1o[:, 0:40], b5t_sb[:])
            th = cp.tile([128, 32], F32)
            nc.scalar.activation(th[:], prefull[:, 0:32], AF.Tanh, scale=0.5)
            gates = cp.tile([128, 32], F32)
            nc.vector.tensor_scalar(gates[:], th[:], 0.5, 0.5, OP.mult, OP.add)
            cnew = cp.tile([128, 8], F32)
            nc.scalar.activation(cnew[:], prefull[:, 32:40], AF.Tanh)
            S_all = cp.tile([128, 1], F32)
            nc.gpsimd.partition_all_reduce(
                S_all[:], stage1o[:, 40:41], 128,
                bass.bass_isa.ReduceOp.add)
            invS = cp.tile([128, 1], F32)
            nc.vector.reciprocal(invS[:], S_all[:])
            t1 = cp.tile([128, 8], F32)
            nc.vector.tensor_mul(t1[:], gates[:, 0:8], c2t_sb[:])
            t2 = cp.tile([128, 8], F32)
            nc.vector.tensor_mul(t2[:], gates[:, 8:16], cnew[:])
            ct0 = cp.tile([128, 8], F32)
            nc.vector.tensor_add(ct0[:], t1[:], t2[:])

            stage2o = cp.tile([128, 8], F32)
            nc.sync.dma_start(stage2o[:], cc2_out[:])

            # ---- LSTM tail --------------------------------------------
            mt_sb = cp.tile([128, 8], F32)
            nc.scalar.activation(mt_sb[:], stage2o[:], AF.Tanh,
                                 scale=invS[:, 0:1])
            t3 = cp.tile([128, 8], F32)
            nc.vector.tensor_mul(t3[:], gates[:, 24:32], mt_sb[:])
            ct = cp.tile([128, 8], F32)
            nc.vector.tensor_add(ct[:], ct0[:], t3[:])
            tct = cp.tile([128, 8], F32)
            nc.scalar.activation(tct[:], ct[:], AF.Tanh)
            ht = cp.tile([128, 8], F32)
            nc.vector.tensor_mul(ht[:], gates[:, 16:24], tct[:])
            ht_r = cp.tile([128, 8], BF16)
            nc.vector.tensor_copy(ht_r[:], ht[:])

            # ---- A2C head: hh = relu(W_ih @ h_t + b_ih) ---------------
            # moving-operand form: p0/p1 banks reused, 16 N=512 matmuls
            for c in range(8):
                nc.tensor.matmul(p0[:], ht_r[:, c:c + 1],
                                 wiht_sb[:, c, 0:512],
                                 start=(c == 0), stop=(c == 7))
                nc.tensor.matmul(p1[:], ht_r[:, c:c + 1],
                                 wiht_sb[:, c, 512:1024],
                                 start=(c == 0), stop=(c == 7))
            hh_row = cp.tile([1, H], F32)
            nc.vector.tensor_copy(hh_row[0:1, 0:512], p0[:])
            nc.vector.tensor_copy(hh_row[0:1, 512:1024], p1[:])
            for n in range(8):
                nc.tensor.transpose(psum_mt[:, n:n + 1],
                                    hh_row[0:1, n * 128:(n + 1) * 128],
                                    ones_sb[0:1, 0:1])
            hhb_sb = cp.tile([128, 8], F32)
            nc.vector.tensor_add(hhb_sb[:], psum_mt[:], biht_sb[:])
            hh_sb = cp.tile([128, 8], F32)
            nc.scalar.activation(hh_sb[:], hhb_sb[:], AF.Relu)

            psum_av = pp.tile([1, 3], F32, tag="pre0")
            for c in range(8):
                nc.tensor.matmul(psum_av[:], hh_sb[:, c:c + 1],
                                 wact_sb[:, c * 3:(c + 1) * 3],
                                 start=(c == 0), stop=(c == 7))
            av = cp.tile([1, 3], F32)
            nc.vector.tensor_add(av[:], psum_av[:], bac_sb[:])

            # ---- outputs ----------------------------------------------
            out_sb = cp.tile([128, 16], F32)
            nc.vector.tensor_copy(out_sb[:, 0:8], ht[:])
            nc.vector.tensor_copy(out_sb[:, 8:16], ct[:])
            nc.sync.dma_start(out_hc[:], out_sb[:])
            nc.sync.dma_start(out_av[:], av[:])

    nc.compile()
    return nc


def _get_nc():
    if "nc" not in _CACHE:
        _CACHE["nc"] = _build()
    return _CACHE["nc"]


def _prep_in_maps(x_t, h, c, keys, vals, W_i2h, b_i2h, W_h2h, b_h2h,
                  W_ih, b_ih, W_actor, b_actor, W_critic, b_critic, pick_arm):
    f = np.float32
    x_t = np.asarray(x_t, f)
    h = np.asarray(h, f).reshape(-1)          # [H]
    c = np.asarray(c, f).reshape(-1)          # [H]
    keys = np.asarray(keys, f)
    vals = np.asarray(vals, f)

    pa = int(np.asarray(pick_arm))
    start = min(max(pa * RD, 0), IN_DIM - RD)  # jax dynamic_slice clamping
    q = x_t[0, start:start + RD]

    q_rep = np.ascontiguousarray(
        np.broadcast_to(np.tile(q, G), (128, G * RD)))

    b5 = (np.asarray(b_i2h, f) + np.asarray(b_h2h, f))
    b5t = np.ascontiguousarray(b5.reshape(40, 128).T)
    biht = np.ascontiguousarray(np.asarray(b_ih, f).reshape(8, 128).T)
    c2t = np.ascontiguousarray(c.reshape(8, 128).T)

    BF = ml_dtypes.bfloat16
    wiht = np.ascontiguousarray(
        np.asarray(W_ih, f).T.reshape(8, 128, H).transpose(1, 0, 2)
        .reshape(128, 8 * H)).astype(BF)
    wac = np.vstack([np.asarray(W_actor, f), np.asarray(W_critic, f)])  # [3,H]
    wact = np.ascontiguousarray(
        wac.T.reshape(8, 128, 3).transpose(1, 0, 2).reshape(128, 24))
    bac = np.concatenate([np.asarray(b_actor, f),
                          np.asarray(b_critic, f)]).reshape(1, 3)

    W_i2hT = np.ascontiguousarray(np.asarray(W_i2h, f).T)
    wxt_zero = np.zeros_like(W_i2hT)
    x_col = np.ascontiguousarray(x_t[0].reshape(IN_DIM, 1))
    x_zero = np.zeros_like(x_col)

    in_maps = []
    for k in range(N_CORES):
        r0 = k * PER
        r1 = min(r0 + PER, D)
        n_valid = r1 - r0

        vals_p = np.zeros((PER, H), f)
        vals_p[:n_valid] = vals[r0:r1]
        vals_s = np.ascontiguousarray(
            vals_p.reshape(G, 128, H).transpose(1, 0, 2)
            .reshape(128, G * H)).astype(BF)
        keys_p = np.zeros((PER, RD), f)
        keys_p[:n_valid] = keys[r0:r1]
        keys_t = np.ascontiguousarray(
            keys_p.reshape(G, 128, RD).transpose(1, 0, 2).reshape(128, G * RD))
        idx = np.arange(G)[None, :] * 128 + np.arange(128)[:, None]
        mask = (idx < n_valid).astype(f)

        wht = np.ascontiguousarray(
            np.asarray(W_h2h, f)[:, k * 128:(k + 1) * 128].T)
        h_col = np.ascontiguousarray(h[k * 128:(k + 1) * 128].reshape(128, 1))

        in_maps.append({
            "vals_s": vals_s,
            "keys_t": keys_t,
            "q_rep": q_rep,
            "mask": mask,
            "wht": wht,
            "wxt": W_i2hT if k == 0 else wxt_zero,
            "x_col": x_col if k == 0 else x_zero,
            "h_col": h_col,
            "c2t": c2t,
            "b5t": b5t,
            "biht": biht,
            "wiht": wiht,
            "wact": wact,
            "bac": bac,
        })
    return in_maps


def _postprocess(out_hc, out_av):
    h_t = np.ascontiguousarray(out_hc[:, 0:8].T).reshape(-1)
    c_t = np.ascontiguousarray(out_hc[:, 8:16].T).reshape(-1)
    logits = out_av[0, 0:2].astype(np.float32)
    v = np.float32(out_av[0, 2])
    m = logits.max()
    ex = np.exp(logits - m)
    pi = (ex / ex.sum()).astype(np.float32)
    a = int(np.argmax(np.log(pi) + GUMBEL))
    logp = np.float32(np.log(pi[a]))
    return np.concatenate([pi, [v], [logp], h_t, c_t]).astype(np.float32)


def kernel(**inputs) -> np.ndarray:
    nc = _get_nc()
    in_maps = _prep_in_maps(**inputs)
    res = run_bass_kernel_spmd(
        nc, in_maps, core_ids=list(range(N_CORES)),
        **_CACHE.get("run_kwargs", {}))
    _CACHE["last_results"] = res
    r0 = res.results[0]
    return _postprocess(r0["out_hc"], r0["out_av"])


# revision 18
# speedup vs baseline: 1.7076x; 1.0236x over previous
"""Trainium2 Bass kernel for CompositionalTwoArmedAgent (DND-LSTM A2C step).

Strategy (8 NeuronCores, SPMD + AllReduce):
  - DND keys/vals tables sharded row-wise: 12544 rows/core (core 7 padded).
  - Cosine similarities are bounded in [-1, 1], so the softmax needs no
    max pass: each core computes e_i = exp(cos_i), a partial sum S_k and a
    partial weighted value sum p_k = e @ vals_k (TensorE, fp32r full rate).
  - The LSTM i2h/h2h GEMM is sharded over its contraction dim (128 h-dims
    per core; the x_t @ W_i2h.T part is zero-fed on cores 1..7).
  - Two AllReduces: [preact(5120) | S(1)] early (hidden under the vals
    stream, so the gate math is precomputed), p(1024) late (4 KB).
  - Every core then computes the identical tiny LSTM/A2C tail; host reads
    core 0's output, applies the 2-class softmax / fixed-key categorical
    sample, and packs the reference's output layout.
"""

import ml_dtypes
import numpy as np

import concourse.bacc as bacc
import concourse.bass as bass
import concourse.mybir as mybir
import concourse.tile as tile
from concourse.bass_utils import run_bass_kernel_spmd

N_CORES = 8
D, RD, H, IN_DIM, A = 100000, 10, 1024, 14, 2
PER = 12544            # padded rows per core = G * 128
G = 98                 # 128-row chunks per core
BLOCKS = [14] * 6 + [7, 4, 2, 1]   # chunks per vals DMA block (descending tail)
F32 = mybir.dt.float32
F32R = mybir.dt.float32r
BF16 = mybir.dt.bfloat16

# jax.random.gumbel(jax.random.key(1), (2,), float32) — fixed constants of the
# reference's categorical sample (verified against jax.random.categorical).
GUMBEL = np.array([0.5325072, -0.01641824], np.float32)

_CACHE = {}


def _input_specs():
    return [
        ("vals_s", [128, G * H], BF16),    # row-chunk-tiled vals shard
        ("keys_t", [128, G * RD], F32),
        ("q_rep", [128, G * RD], F32),
        ("mask", [128, G], F32),
        ("wht", [128, 5 * H], F32R),
        ("wxt", [IN_DIM, 5 * H], F32R),
        ("x_col", [IN_DIM, 1], F32R),
        ("h_col", [128, 1], F32R),
        ("c2t", [128, 8], F32),
        ("b5t", [128, 40], F32),
        ("biht", [128, 8], F32),
        ("wiht", [128, 8 * H], BF16),
        ("wact", [128, 24], F32),
        ("bac", [1, 3], F32),
    ]


def _build():
    nc = bacc.Bacc("TRN2", target_bir_lowering=False, debug=False,
                   num_devices=N_CORES)
    d = {name: nc.dram_tensor(name, shp, dt, kind="ExternalInput")
         for name, shp, dt in _input_specs()}
    out_hc = nc.dram_tensor("out_hc", [128, 16], F32, kind="ExternalOutput")
    out_av = nc.dram_tensor("out_av", [1, 3], F32, kind="ExternalOutput")

    AF = mybir.ActivationFunctionType
    OP = mybir.AluOpType

    with tile.TileContext(nc) as tc:
        with (
            tc.tile_pool(name="const", bufs=1) as cp,
            tc.tile_pool(name="vals", bufs=3) as vp,
            tc.tile_pool(name="ps", bufs=1, space="PSUM") as pp,
            tc.tile_pool(name="dram", bufs=1, space="DRAM") as dp,
        ):
            # ---- persistent loads -------------------------------------
            keys_sb = cp.tile([128, G * RD], F32)
            q_sb = cp.tile([128, G * RD], F32)
            mask_sb = cp.tile([128, G], F32)
            wht_sb = cp.tile([128, 5 * H], F32R)
            wxt_sb = cp.tile([IN_DIM, 5 * H], F32R)
            x_col_sb = cp.tile([IN_DIM, 1], F32R)
            h_col_sb = cp.tile([128, 1], F32R)
            c2t_sb = cp.tile([128, 8], F32)
            b5t_sb = cp.tile([128, 40], F32)
            biht_sb = cp.tile([128, 8], F32)
            wiht_sb = cp.tile([128, 8, H], BF16)
            wact_sb = cp.tile([128, 24], F32)
            bac_sb = cp.tile([1, 3], F32)
            for name, t in [("wht", wht_sb), ("wxt", wxt_sb),
                            ("x_col", x_col_sb), ("h_col", h_col_sb),
                            ("keys_t", keys_sb), ("q_rep", q_sb),
                            ("mask", mask_sb), ("c2t", c2t_sb),
                            ("b5t", b5t_sb), ("biht", biht_sb),
                            ("wact", wact_sb), ("bac", bac_sb)]:
                nc.scalar.dma_start(t[:], d[name][:])
            nc.scalar.dma_start(
                wiht_sb[:], d["wiht"][:].rearrange("p (c j) -> p c j", j=H))

            ones_sb = cp.tile([128, 128], F32)
            nc.vector.memset(ones_sb[:], 1.0)

            # ---- ||q||^2 broadcast to all partitions ------------------
            sq_q = cp.tile([1, RD], F32)
            nc.scalar.activation(sq_q[:], q_sb[0:1, 0:RD], AF.Square)
            qnsq = cp.tile([1, 1], F32)
            nc.vector.reduce_sum(qnsq[:], sq_q[:], axis=mybir.AxisListType.X)
            psum_qn = pp.tile([128, 1], F32, tag="ps_small")
            nc.tensor.matmul(psum_qn[:], ones_sb[0:1, :], qnsq[:])
            qn2b = cp.tile([128, 1], F32)
            nc.vector.tensor_copy(qn2b[:], psum_qn[:])

            # ---- cosine sims -> masked exp weights --------------------
            prod = cp.tile([128, G * RD], F32)
            nc.vector.tensor_mul(prod[:], keys_sb[:], q_sb[:])
            dots = cp.tile([128, G], F32)
            nc.vector.tensor_reduce(
                dots[:], prod[:].rearrange("p (g r) -> p g r", r=RD),
                axis=mybir.AxisListType.X, op=OP.add)
            sqk = cp.tile([128, G * RD], F32)
            nc.scalar.activation(sqk[:], keys_sb[:], AF.Square)
            nsq = cp.tile([128, G], F32)
            nc.vector.tensor_reduce(
                nsq[:], sqk[:].rearrange("p (g r) -> p g r", r=RD),
                axis=mybir.AxisListType.X, op=OP.add)
            d2 = cp.tile([128, G], F32)
            nc.vector.tensor_scalar(d2[:], nsq[:], qn2b[:, 0:1], None, OP.mult)
            den = cp.tile([128, G], F32)
            nc.scalar.activation(den[:], d2[:], AF.Sqrt)
            denc = cp.tile([128, G], F32)
            nc.vector.tensor_scalar_max(denc[:], den[:], 1e-8)
            rec = cp.tile([128, G], F32)
            nc.vector.reciprocal(rec[:], denc[:])
            s_sb = cp.tile([128, G], F32)
            nc.vector.tensor_mul(s_sb[:], dots[:], rec[:])
            eraw = cp.tile([128, G], F32)
            nc.scalar.activation(eraw[:], s_sb[:], AF.Exp)
            e_sb = cp.tile([128, G], F32)
            rowsum = cp.tile([128, 1], F32)
            nc.vector.scalar_tensor_tensor(
                e_sb[:], eraw[:], 1.0, mask_sb[:], OP.mult, OP.mult,
                accum_out=rowsum[:])
            e_r = cp.tile([128, G], BF16)
            nc.vector.tensor_copy(e_r[:], e_sb[:])

            # ---- preact partial: [x;h_chunk] @ [WxT;WhT] --------------
            # moving-operand fp32r form: 20 N=512 matmuls into [1,512] rows,
            # then 40 PE transposes back to the compact [128, 40] col layout.
            psum_pre = pp.tile([128, 40], F32)
            for n in range(10):
                pre_ps = pp.tile([1, 512], F32, tag=f"pre{n % 2}")
                nc.tensor.matmul(pre_ps[:], h_col_sb[:],
                                 wht_sb[:, n * 512:(n + 1) * 512],
                                 start=True, stop=False)
                nc.tensor.matmul(pre_ps[:], x_col_sb[:],
                                 wxt_sb[:, n * 512:(n + 1) * 512],
                                 start=False, stop=True)
                row_scr = cp.tile([1, 512], F32, tag="rowscr", bufs=2)
                nc.vector.tensor_copy(row_scr[:], pre_ps[:])
                for t in range(4):
                    nc.tensor.transpose(psum_pre[:, 4 * n + t:4 * n + t + 1],
                                        row_scr[0:1, t * 128:(t + 1) * 128],
                                        ones_sb[0:1, 0:1])

            # ---- AllReduce #1: [preact(40) | S(1)] (hidden) -----------
            stage1 = cp.tile([128, 41], F32)
            nc.vector.tensor_copy(stage1[:, 0:40], psum_pre[:])
            nc.vector.tensor_copy(stage1[:, 40:41], rowsum[:])
            cc1_in = dp.tile([128, 41], F32)
            cc1_out = dp.tile([128, 41], F32, addr_space="Shared")
            nc.gpsimd.dma_start(cc1_in[:], stage1[:])
            nc.gpsimd.collective_compute(
                "AllReduce", OP.add,
                replica_groups=[list(range(N_CORES))],
                ins=[cc1_in[:]], outs=[cc1_out[:]])
            stage1o = cp.tile([128, 41], F32)
            nc.gpsimd.dma_start(stage1o[:], cc1_out[:])

            # ---- big matvec: p = e @ vals (fp32r, streamed) -----------
            p0 = pp.tile([1, 512], F32)
            p1 = pp.tile([1, 512], F32)
            g = 0
            for nb in BLOCKS:
                v = vp.tile([128, nb, H], BF16, tag="v")
                src = d["vals_s"][:, g * H:(g + nb) * H]
                nc.sync.dma_start(v[:], src.rearrange("p (c h) -> p c h", h=H))
                for c in range(nb):
                    e_col = e_r[:, g:g + 1]
                    nc.tensor.matmul(p0[:], e_col, v[:, c, 0:512],
                                     start=(g == 0), stop=(g == G - 1))
                    nc.tensor.matmul(p1[:], e_col, v[:, c, 512:1024],
                                     start=(g == 0), stop=(g == G - 1))
                    g += 1

            # ---- transpose p to [128, 8] ------------------------------
            p_sb = cp.tile([1, H], F32)
            nc.vector.tensor_copy(p_sb[0:1, 0:512], p0[:])
            nc.vector.tensor_copy(p_sb[0:1, 512:1024], p1[:])
            psum_mt = pp.tile([128, 8], F32)
            for n in range(8):
                nc.tensor.transpose(psum_mt[:, n:n + 1],
                                    p_sb[0:1, n * 128:(n + 1) * 128],
                                    ones_sb[0:1, 0:1])

            # ---- AllReduce #2: p (4 KB) -------------------------------
            stage2 = cp.tile([128, 8], F32)
            i_stage2 = nc.vector.tensor_copy(stage2[:], psum_mt[:])
            cc2_in = dp.tile([128, 8], F32)
            cc2_out = dp.tile([128, 8], F32, addr_space="Shared")
            nc.sync.dma_start(cc2_in[:], stage2[:])
            i_cc2 = nc.gpsimd.collective_compute(
                "AllReduce", OP.add,
                replica_groups=[list(range(N_CORES))],
                ins=[cc2_in[:]], outs=[cc2_out[:]])
            # ---- gate math from AR1 (hidden under the vals stream) ----
            prefull = cp.tile([128, 40], F32)
            i_pf = nc.vector.tensor_add(prefull[:], stage1o[:, 0:40], b5t_sb[:])
            # keep the AR1-gated DVE chain behind the AR2 staging copy so the
            # scheduler cannot stall the vector queue on AR1 completion
            tile.add_dep_helper(i_pf.ins, i_stage2.ins, sync=False,
                                reason="gate math after AR2 staging")
            th = cp.tile([128, 32], F32)
            nc.scalar.activation(th[:], prefull[:, 0:32], AF.Tanh, scale=0.5)
            gates = cp.tile([128, 32], F32)
            nc.vector.tensor_scalar(gates[:], th[:], 0.5, 0.5, OP.mult, OP.add)
            cnew = cp.tile([128, 8], F32)
            nc.scalar.activation(cnew[:], prefull[:, 32:40], AF.Tanh)
            S_all = cp.tile([128, 1], F32)
            i_sall = nc.gpsimd.partition_all_reduce(
                S_all[:], stage1o[:, 40:41], 128,
                bass.bass_isa.ReduceOp.add)
            tile.add_dep_helper(i_sall.ins, i_cc2.ins, sync=False,
                                reason="keep gpsimd doorbell ahead of S reduce")
            invS = cp.tile([128, 1], F32)
            nc.vector.reciprocal(invS[:], S_all[:])
            t1 = cp.tile([128, 8], F32)
            nc.vector.tensor_mul(t1[:], gates[:, 0:8], c2t_sb[:])
            t2 = cp.tile([128, 8], F32)
            nc.vector.tensor_mul(t2[:], gates[:, 8:16], cnew[:])
            ct0 = cp.tile([128, 8], F32)
            nc.vector.tensor_add(ct0[:], t1[:], t2[:])

            stage2o = cp.tile([128, 8], F32)
            nc.sync.dma_start(stage2o[:], cc2_out[:])

            # ---- LSTM tail --------------------------------------------
            mt_sb = cp.tile([128, 8], F32)
            nc.scalar.activation(mt_sb[:], stage2o[:], AF.Tanh,
                                 scale=invS[:, 0:1])
            t3 = cp.tile([128, 8], F32)
            nc.vector.tensor_mul(t3[:], gates[:, 24:32], mt_sb[:])
            ct = cp.tile([128, 8], F32)
            nc.vector.tensor_add(ct[:], ct0[:], t3[:])
            tct = cp.tile([128, 8], F32)
            nc.scalar.activation(tct[:], ct[:], AF.Tanh)
            ht = cp.tile([128, 8], F32)
            nc.vector.tensor_mul(ht[:], gates[:, 16:24], tct[:])
            ht_r = cp.tile([128, 8], BF16)
            nc.vector.tensor_copy(ht_r[:], ht[:])

            # ---- A2C head: hh = relu(W_ih @ h_t + b_ih) ---------------
            # moving-operand form: p0/p1 banks reused, 16 N=512 matmuls
            for c in range(8):
                nc.tensor.matmul(p0[:], ht_r[:, c:c + 1],
                                 wiht_sb[:, c, 0:512],
                                 start=(c == 0), stop=(c == 7))
                nc.tensor.matmul(p1[:], ht_r[:, c:c + 1],
                                 wiht_sb[:, c, 512:1024],
                                 start=(c == 0), stop=(c == 7))
            hh_row = cp.tile([1, H], F32)
            nc.vector.tensor_copy(hh_row[0:1, 0:512], p0[:])
            nc.vector.tensor_copy(hh_row[0:1, 512:1024], p1[:])
            for n in range(8):
                nc.tensor.transpose(psum_mt[:, n:n + 1],
                                    hh_row[0:1, n * 128:(n + 1) * 128],
                                    ones_sb[0:1, 0:1])
            hhb_sb = cp.tile([128, 8], F32)
            nc.vector.tensor_add(hhb_sb[:], psum_mt[:], biht_sb[:])
            hh_sb = cp.tile([128, 8], F32)
            nc.scalar.activation(hh_sb[:], hhb_sb[:], AF.Relu)

            psum_av = pp.tile([1, 3], F32, tag="pre0")
            for c in range(8):
                nc.tensor.matmul(psum_av[:], hh_sb[:, c:c + 1],
                                 wact_sb[:, c * 3:(c + 1) * 3],
                                 start=(c == 0), stop=(c == 7))
            av = cp.tile([1, 3], F32)
            nc.vector.tensor_add(av[:], psum_av[:], bac_sb[:])

            # ---- outputs ----------------------------------------------
            out_sb = cp.tile([128, 16], F32)
            nc.vector.tensor_copy(out_sb[:, 0:8], ht[:])
            nc.vector.tensor_copy(out_sb[:, 8:16], ct[:])
            nc.sync.dma_start(out_hc[:], out_sb[:])
            nc.sync.dma_start(out_av[:], av[:])

    nc.compile()
    return nc


def _get_nc():
    if "nc" not in _CACHE:
        _CACHE["nc"] = _build()
    return _CACHE["nc"]


def _prep_in_maps(x_t, h, c, keys, vals, W_i2h, b_i2h, W_h2h, b_h2h,
                  W_ih, b_ih, W_actor, b_actor, W_critic, b_critic, pick_arm):
    f = np.float32
    x_t = np.asarray(x_t, f)
    h = np.asarray(h, f).reshape(-1)          # [H]
    c = np.asarray(c, f).reshape(-1)          # [H]
    keys = np.asarray(keys, f)
    vals = np.asarray(vals, f)

    pa = int(np.asarray(pick_arm))
    start = min(max(pa * RD, 0), IN_DIM - RD)  # jax dynamic_slice clamping
    q = x_t[0, start:start + RD]

    q_rep = np.ascontiguousarray(
        np.broadcast_to(np.tile(q, G), (128, G * RD)))

    b5 = (np.asarray(b_i2h, f) + np.asarray(b_h2h, f))
    b5t = np.ascontiguousarray(b5.reshape(40, 128).T)
    biht = np.ascontiguousarray(np.asarray(b_ih, f).reshape(8, 128).T)
    c2t = np.ascontiguousarray(c.reshape(8, 128).T)

    BF = ml_dtypes.bfloat16
    wiht = np.ascontiguousarray(
        np.asarray(W_ih, f).T.reshape(8, 128, H).transpose(1, 0, 2)
        .reshape(128, 8 * H)).astype(BF)
    wac = np.vstack([np.asarray(W_actor, f), np.asarray(W_critic, f)])  # [3,H]
    wact = np.ascontiguousarray(
        wac.T.reshape(8, 128, 3).transpose(1, 0, 2).reshape(128, 24))
    bac = np.concatenate([np.asarray(b_actor, f),
                          np.asarray(b_critic, f)]).reshape(1, 3)

    W_i2hT = np.ascontiguousarray(np.asarray(W_i2h, f).T)
    wxt_zero = np.zeros_like(W_i2hT)
    x_col = np.ascontiguousarray(x_t[0].reshape(IN_DIM, 1))
    x_zero = np.zeros_like(x_col)

    in_maps = []
    for k in range(N_CORES):
        r0 = k * PER
        r1 = min(r0 + PER, D)
        n_valid = r1 - r0

        vals_p = np.zeros((PER, H), f)
        vals_p[:n_valid] = vals[r0:r1]
        vals_s = np.ascontiguousarray(
            vals_p.reshape(G, 128, H).transpose(1, 0, 2)
            .reshape(128, G * H)).astype(BF)
        keys_p = np.zeros((PER, RD), f)
        keys_p[:n_valid] = keys[r0:r1]
        keys_t = np.ascontiguousarray(
            keys_p.reshape(G, 128, RD).transpose(1, 0, 2).reshape(128, G * RD))
        idx = np.arange(G)[None, :] * 128 + np.arange(128)[:, None]
        mask = (idx < n_valid).astype(f)

        wht = np.ascontiguousarray(
            np.asarray(W_h2h, f)[:, k * 128:(k + 1) * 128].T)
        h_col = np.ascontiguousarray(h[k * 128:(k + 1) * 128].reshape(128, 1))

        in_maps.append({
            "vals_s": vals_s,
            "keys_t": keys_t,
            "q_rep": q_rep,
            "mask": mask,
            "wht": wht,
            "wxt": W_i2hT if k == 0 else wxt_zero,
            "x_col": x_col if k == 0 else x_zero,
            "h_col": h_col,
            "c2t": c2t,
            "b5t": b5t,
            "biht": biht,
            "wiht": wiht,
            "wact": wact,
            "bac": bac,
        })
    return in_maps


def _postprocess(out_hc, out_av):
    h_t = np.ascontiguousarray(out_hc[:, 0:8].T).reshape(-1)
    c_t = np.ascontiguousarray(out_hc[:, 8:16].T).reshape(-1)
    logits = out_av[0, 0:2].astype(np.float32)
    v = np.float32(out_av[0, 2])
    m = logits.max()
    ex = np.exp(logits - m)
    pi = (ex / ex.sum()).astype(np.float32)
    a = int(np.argmax(np.log(pi) + GUMBEL))
    logp = np.float32(np.log(pi[a]))
    return np.concatenate([pi, [v], [logp], h_t, c_t]).astype(np.float32)


def kernel(**inputs) -> np.ndarray:
    nc = _get_nc()
    in_maps = _prep_in_maps(**inputs)
    res = run_bass_kernel_spmd(
        nc, in_maps, core_ids=list(range(N_CORES)),
        **_CACHE.get("run_kwargs", {}))
    _CACHE["last_results"] = res
    r0 = res.results[0]
    return _postprocess(r0["out_hc"], r0["out_av"])
